# revision 52
# baseline (speedup 1.0000x reference)
"""BasicTransformerBlock on 8 TRN2 NeuronCores.

Sharding: data-parallel, core = (batch b in 0..3) x (sequence half h in 0..1).
Each core receives its batch element's full sequence rotated so its local 512
rows come first (softmax over keys is permutation invariant), computes K/V of
attn1 for all 1024 tokens (duplicated across the pair, ~10% extra FLOPs, zero
collectives), and everything else for its 512 local tokens only.

On-chip layout: feature-major activations [features on partitions, tokens on
free axis]. The residual stream, LN math and PSUM accumulation stay fp32.
LayerNorm partition reductions and per-token broadcasts use fp16 ones-matmuls.
Attention softmax denominators come free from a ones-column appended to V.

Precision: Q/K/V/O projections (both attns), attn1's attnV and the FF
down-proj run in fp8 e4m3 with DoubleRow matmuls (two 128-row contraction
chunks per pass, 2x PE rate); weights are pre-scaled x32 on the host and the
PSUM rescaled by 1/32 at evacuation. The GEGLU up-proj (the dominant error
path) stays fp16. Attention scores use fp8 operands at the normal rate.

Schedule: Q/K projections run one head-pair ahead of the score matmuls inside
a single fused LN1+proj+attention phase, so the PE streams continuously while
ACT chases with exp. LN2/LN3 statistics are inlined into the O-projection
consume loops. K2/V2 run at the attn1 tail; FF weights stream in early on
wide rolling pools. SBUF pools use the queue (ring) allocator so each pool
releases as soon as its contents die.
"""

import sys
import types

sys.path.insert(0, "/opt/trn_rl_repo")

# concourse fetches the NTFF profile hook from antenv.axon_hooks, which the
# agent image's antenv stub lacks. Register a shim so trace=True works.
if "antenv.axon_hooks" not in sys.modules:
    _hooks = types.ModuleType("antenv.axon_hooks")
    _HOOK = [None]

    def _get_hook():
        if _HOOK[0] is None:
            try:
                from trn_agent_boot.trn_boot import _ntff_profile_via_ctypes

                _HOOK[0] = _ntff_profile_via_ctypes("/opt/axon/libaxon_pjrt.so")
            except Exception:
                _HOOK[0] = None
        return _HOOK[0]

    _hooks.get_axon_ntff_profile_hook = _get_hook
    _hooks.set_axon_ntff_profile_hook = lambda h: _HOOK.__setitem__(0, h)
    sys.modules["antenv.axon_hooks"] = _hooks
    try:
        import antenv

        antenv.axon_hooks = _hooks
    except ImportError:
        pass

import ml_dtypes
import numpy as np

import concourse.bass as bass
import concourse.mybir as mybir
import concourse.tile as tile
from concourse import bacc, bass_utils

dt = mybir.dt
F32, F16, F8 = dt.float32, dt.float16, dt.float8e4
AF = mybir.ActivationFunctionType
DR = mybir.MatmulPerfMode.DoubleRow
MUL, ADD, SUB = mybir.AluOpType.mult, mybir.AluOpType.add, mybir.AluOpType.subtract

DIM, HEADS, DHEAD, CTX_DIM, DFF = 1280, 20, 64, 768, 5120
BATCH, NTOK, MCTX = 4, 1024, 77
EPS = 1e-5
SCALE = DHEAD ** -0.5
N_CORES = 8
T = 512         # local tokens per core
TKV = 1024      # attn1 key/value tokens per core
KC = DIM // 128           # 10
KCP = KC // 2             # 5 contraction pairs
KCX = CTX_DIM // 128      # 6
KCXP = KCX // 2           # 3
JFF = DFF // 128          # 40 (chunks of the gated hidden)
JFP = JFF // 2            # 20 pairs for the down-proj contraction
P = 128
WS = 32.0                 # fp8 weight pre-scale (power of two)
WINV = 1.0 / WS
VPAD = 68                 # DHEAD+1 padded so the V pair stride is 16B aligned
MPAD = 80                 # MCTX padded likewise for the ctx pair stride

last_exec_time_ns = None


def _emit(tc, d, trivial_aff, trivial_bias):
    nc = tc.nc

    const = tc.alloc_tile_pool(name="const", bufs=1)
    ones_col = const.tile([P, 1], F16, name="ones_col")
    nc.vector.memset(ones_col[:], 1.0)
    ones_row = const.tile([1, P], F16, name="ones_row")
    nc.vector.memset(ones_row[:], 1.0)
    if not trivial_aff:
        aff = const.tile([P, 60], F32, name="aff")
        nc.sync.dma_start(aff[:], d["aff"])
    if not trivial_bias:
        biases = const.tile([P, 110], F32, name="biases")
        nc.sync.dma_start(biases[:], d["biases"])

    tmp = tc.alloc_tile_pool(name="tmp", bufs=1)

    # ---------------- helpers ----------------

    def bias_ap(col):
        return biases[:, col:col + 1]

    def ln_stats_mm(xh_ap, xsq, sums_ps, sq_ps, start, stop):
        nc.tensor.matmul(sums_ps[:], ones_col[:], xh_ap, start=start, stop=stop)
        nc.tensor.matmul(sq_ps[:], ones_col[:], xsq[:], start=start, stop=stop)

    def ln_finalize(ln_idx, t, psum_p, sums_ps, sq_ps, out_fn):
        """Turn accumulated sum / sum-sq PSUMs into mu/rstd and emit the
        normalized outputs via out_fn(c, mu16, rstd16)."""
        ssum = tmp.tile([1, 512], F16, name=f"ssum{ln_idx}_{t}", tag="ssum", bufs=2)
        nc.vector.tensor_copy(out=ssum[:], in_=sums_ps[:])
        ssq = tmp.tile([1, 512], F16, name=f"ssq{ln_idx}_{t}", tag="ssq", bufs=2)
        nc.vector.tensor_copy(out=ssq[:], in_=sq_ps[:])
        bs_ps = psum_p.tile([P, 512], F32, name=f"bs{ln_idx}_{t}", tag="proj", bufs=2)
        nc.tensor.matmul(bs_ps[:], ones_row[:], ssum[:], start=True, stop=True)
        bq_ps = psum_p.tile([P, 512], F32, name=f"bq{ln_idx}_{t}", tag="proj", bufs=2)
        nc.tensor.matmul(bq_ps[:], ones_row[:], ssq[:], start=True, stop=True)
        mu16 = tmp.tile([P, 512], F16, name=f"mu16_{ln_idx}_{t}", tag="mu16", bufs=2)
        nc.vector.tensor_scalar_mul(mu16[:], bs_ps[:], 1.0 / DIM)
        musq = tmp.tile([P, 512], F16, name=f"musq{ln_idx}_{t}", tag="musq", bufs=1)
        nc.vector.tensor_mul(musq[:], mu16[:], mu16[:])
        # musq - EPS, so var = ex2 - musq + EPS below
        nc.vector.tensor_scalar_sub(musq[:], musq[:], EPS)
        var = tmp.tile([P, 512], F16, name=f"var{ln_idx}_{t}", tag="var", bufs=1)
        nc.vector.scalar_tensor_tensor(var[:], bq_ps[:], 1.0 / DIM, musq[:], MUL, SUB)
        std = tmp.tile([P, 512], F32, name=f"std{ln_idx}_{t}", tag="std", bufs=1)
        nc.scalar.sqrt(std[:], var[:])
        rstd = tmp.tile([P, 512], F32, name=f"rstd{ln_idx}_{t}", tag="rstd", bufs=1)
        nc.vector.reciprocal_approx_fast(rstd[:], std[:])
        rstd16 = tmp.tile([P, 512], F16, name=f"rstd16_{ln_idx}_{t}", tag="rstd16",
                          bufs=2)
        nc.vector.tensor_copy(out=rstd16[:], in_=rstd[:])
        for c in range(KC):
            out_fn(c, mu16, rstd16)

    def ln_out(ln_idx, t, c, src_ap, mu16, rstd16, out_ap, eng=None):
        """out = (src - mu) * rstd (* g + b)."""
        eng = eng or nc.vector
        xm = tmp.tile([P, 512], F16, name=f"xm{ln_idx}_{t}_{c}", tag="xm", bufs=4)
        eng.tensor_sub(xm[:], src_ap, mu16[:])
        if trivial_aff:
            eng.tensor_mul(out_ap, xm[:], rstd16[:])
        else:
            xn = tmp.tile([P, 512], F16, name=f"xn{ln_idx}_{t}_{c}", tag="xn", bufs=3)
            nc.vector.tensor_mul(xn[:], xm[:], rstd16[:])
            g_ap = aff[:, ln_idx * 20 + c: ln_idx * 20 + c + 1]
            be_ap = aff[:, ln_idx * 20 + 10 + c: ln_idx * 20 + 10 + c + 1]
            xg = tmp.tile([P, 512], F16, name=f"xg{ln_idx}_{t}_{c}", tag="xg", bufs=3)
            nc.vector.tensor_scalar_mul(xg[:], xn[:], g_ap)
            nc.scalar.activation(out_ap, xg[:], AF.Copy, bias=be_ap)

    def attn_finish(head, ops_, out_ap, psum_p, ps_tag, evac_act=False, ps_bufs=2):
        usb = tmp.tile([DHEAD + 1, 512], F16, name=f"usb{head}", tag="usb", bufs=3)
        if evac_act:
            nc.scalar.copy(usb[:], ops_[:])
        else:
            nc.vector.tensor_copy(out=usb[:], in_=ops_[:])
        den = tmp.tile([1, 512], F32, name=f"den{head}", tag="den", bufs=2)
        if evac_act:
            nc.scalar.copy(den[:], ops_[DHEAD:DHEAD + 1, :])
        else:
            nc.vector.tensor_copy(out=den[:], in_=ops_[DHEAD:DHEAD + 1, :])
        rec32 = tmp.tile([1, 512], F32, name=f"rec32_{head}", tag="rec32", bufs=2)
        nc.vector.reciprocal_approx_fast(rec32[:], den[:])
        rec = tmp.tile([1, 512], F16, name=f"rec{head}", tag="rec", bufs=2)
        nc.gpsimd.tensor_copy(out=rec[:], in_=rec32[:])
        bps = psum_p.tile([DHEAD, 512], F32, name=f"bps{head}", tag=ps_tag,
                          bufs=ps_bufs)
        nc.tensor.matmul(bps[:], ones_row[:, :DHEAD], rec[:], start=True, stop=True)
        nc.vector.tensor_mul(out_ap, usb[:DHEAD, :], bps[:])

    # ============ phase A: LN1 + QKV projections + attn1, fused ============
    # Left-side pools open in death-descending order (LIFO release discipline;
    # pools marked * stay open to the end: the queue allocator makes the dead
    # space explicit and cheap).

    # x DMA is the critical path to LN1 — allocate xpool first and emit its
    # DMAs before any weight DMA so the DMA queues serve it first.
    xpool = tc.alloc_tile_pool(name="xpool", bufs=1, side="right")
    x_sb = []
    for c in range(KC):
        xc = xpool.tile([P, TKV], F16, name=f"x_{c}", tag="x", bufs=KC)
        nc.sync.dma_start(xc[:], d["xt"][c * P:(c + 1) * P, :])
        x_sb.append(xc)

    ctxp = tc.alloc_tile_pool(name="ctxp", bufs=1)          # * (tiny)
    ctx8 = []
    for kp in range(KCXP):
        c8 = ctxp.tile([P, 2, MPAD], F8, name=f"ctx8_{kp}", tag="ctx8", bufs=KCXP)
        nc.sync.dma_start(c8[:], d["ctx8"][kp])
        ctx8.append(c8)

    # Residual slice of x; O1 writes x1 into these tiles in place.      *
    resp = tc.alloc_tile_pool(name="resp", bufs=1)
    resid = [resp.tile([P, T], F16, name=f"res_{c}", tag="res", bufs=KC)
             for c in range(KC)]

    o2p = tc.alloc_tile_pool(name="o2p", bufs=1)            # * (attn2 out)
    O2t8 = [o2p.tile([P, 2, T], F8, name=f"o2t8_{i}", tag="o2t8", bufs=KCP)
            for i in range(KCP)]

    k2v2 = tc.alloc_tile_pool(name="k2v2", bufs=1)          # dies at attn2
    K2t = [k2v2.tile([P, MCTX], F16, name=f"k2t_{mc}", tag="k2t", bufs=KC)
           for mc in range(KC)]
    V2t = k2v2.tile([P, HEADS, DHEAD + 1], F16, name="v2t", tag="v2t", bufs=1)

    q2p = tc.alloc_tile_pool(name="q2p", bufs=1)            # dies at attn2
    Q2t = [q2p.tile([P, T], F16, name=f"q2t_{mc}", tag="q2t", bufs=KC)
           for mc in range(KC)]

    wq2p = tc.alloc_tile_pool(name="wq2p", bufs=1)          # dies after Q2
    wq2t = [wq2p.tile([P, KCP, 2, P], F8, name=f"wq2_{mc}", tag="wq2", bufs=KC)
            for mc in range(KC)]

    ln2p = tc.alloc_tile_pool(name="ln2p", bufs=1)          # dies after Q2
    ln28 = [ln2p.tile([P, 2, T], F8, name=f"ln28_{i}", tag="ln28", bufs=KCP)
            for i in range(KCP)]

    wpre = tc.alloc_tile_pool(name="wpre", bufs=1)          # dies after O1
    wo1t = [wpre.tile([P, KCP, 2, P], F8, name=f"wo1_{mc}", tag="wo1", bufs=KC)
            for mc in range(KC)]
    wk2t = [wpre.tile([P, KCXP, 2, P], F8, name=f"wk2_{mc}", tag="wk2", bufs=KC)
            for mc in range(KC)]
    wv2s = [wpre.tile([P, 2, DIM], F8, name=f"wv2_{kp}", tag="wv2", bufs=KCXP)
            for kp in range(KCXP)]

    otp = tc.alloc_tile_pool(name="otp", bufs=1)            # dies after O1
    Ot8 = [otp.tile([P, 2, T], F8, name=f"ot8_{i}", tag="ot8", bufs=KCP)
           for i in range(KCP)]

    wqk = tc.alloc_tile_pool(name="wqk", bufs=1)            # dies at pipe end
    wq1t, wk1t, wv1t = [], [], []
    for mc in range(KC):
        wq1t.append(wqk.tile([P, KCP, 2, P], F8, name=f"wq1_{mc}", tag="wq1", bufs=KC))
        wk1t.append(wqk.tile([P, KCP, 2, P], F8, name=f"wk1_{mc}", tag="wk1", bufs=KC))
    wv1t = [wqk.tile([P, 2, DIM], F8, name=f"wv1_{kp}", tag="wv1", bufs=KCP)
            for kp in range(KCP)]

    ln1p = tc.alloc_tile_pool(name="ln1p", bufs=1)          # dies at pipe end
    ln18 = [ln1p.tile([P, 2, TKV], F8, name=f"ln18_{i}", tag="ln18", bufs=KCP)
            for i in range(KCP)]

    # Weight DMAs, in order of first use (after x so they can't delay LN1).
    for mc in range(KC):
        nc.sync.dma_start(wq1t[mc][:], d["wq1"][mc])
        nc.sync.dma_start(wk1t[mc][:], d["wk1"][mc])
    for kp in range(KCP):
        nc.sync.dma_start(wv1t[kp][:], d["wv1"][kp])
    for c in range(KC):
        nc.sync.dma_start(resid[c][:], d["xt"][c * P:(c + 1) * P, 0:T])
    for mc in range(KC):
        nc.sync.dma_start(wk2t[mc][:], d["wk2"][mc])
    for kp in range(KCXP):
        nc.sync.dma_start(wv2s[kp][:], d["wv2"][kp])
    for mc in range(KC):
        nc.sync.dma_start(wo1t[mc][:], d["wo1"][mc])
    for mc in range(KC):
        nc.sync.dma_start(wq2t[mc][:], d["wq2"][mc])

    # LN1 over all 1024 keys. x arrives fp16 and feeds the stats matmuls
    # directly; both 512-token tiles' stats run back-to-back on the PE, with
    # the normalize work split across DVE and GpSimd.
    ln_psum = tc.alloc_tile_pool(name="ln_psum", bufs=1, space="PSUM")
    stats = []
    for t in range(2):
        sl = slice(t * 512, (t + 1) * 512)
        sums_ps = ln_psum.tile([1, 512], F32, name=f"lns0_{t}", tag="lnstat", bufs=4)
        sq_ps = ln_psum.tile([1, 512], F32, name=f"lnq0_{t}", tag="lnstat", bufs=4)
        for c in range(KC):
            xsq = tmp.tile([P, 512], F16, name=f"xsq0_{t}_{c}", tag="xsq", bufs=3)
            nc.vector.tensor_mul(xsq[:], x_sb[c][:, sl], x_sb[c][:, sl])
            ln_stats_mm(x_sb[c][:, sl], xsq, sums_ps, sq_ps, c == 0, c == KC - 1)
        stats.append((sums_ps, sq_ps))
    for t in range(2):
        sl = slice(t * 512, (t + 1) * 512)

        def ln1_out(c, mu16, rstd16, t=t, sl=sl):
            ln_out(0, t, c, x_sb[c][:, sl], mu16, rstd16,
                   ln18[c // 2][:, c % 2, sl],
                   eng=(nc.vector if c % 2 == 0 else nc.gpsimd))

        ln_finalize(0, t, ln_psum, stats[t][0], stats[t][1], ln1_out)
    xpool.release()
    ln_psum.release()

    sc_psum = tc.alloc_tile_pool(name="sc_psum", bufs=1, space="PSUM")
    ov_psum = tc.alloc_tile_pool(name="ov_psum", bufs=1, space="PSUM")
    pj_psum = tc.alloc_tile_pool(name="pj_psum", bufs=1, space="PSUM")
    qkt = tc.alloc_tile_pool(name="qkt", bufs=1, side="right")
    v8p = tc.alloc_tile_pool(name="v8p", bufs=1, side="right")
    epoolA = tc.alloc_tile_pool(name="epoolA", bufs=4, side="right")
    epoolB = tc.alloc_tile_pool(name="epoolB", bufs=4, side="right")

    Qt = [qkt.tile([P, T], F8, name=f"qt_{mc}", tag="qt", bufs=KC) for mc in range(KC)]
    Kt = [qkt.tile([P, TKV], F8, name=f"kt_{mc}", tag="kt", bufs=KC) for mc in range(KC)]
    V8 = [v8p.tile([P, 2, HEADS, VPAD], F8, name=f"v8_{j}", tag="v8", bufs=4)
          for j in range(4)]

    def qk_proj(c):
        """Project Q chunk c (local tokens) and K chunk c (all 1024 keys)."""
        ps = pj_psum.tile([P, 512], F32, name=f"psq_{c}", tag="proj", bufs=2)
        for kp in range(KCP):
            nc.tensor.matmul(ps[:], wq1t[c][:, kp], ln18[kp][:, :, 0:T],
                             start=(kp == 0), stop=(kp == KCP - 1), perf_mode=DR)
        nc.vector.tensor_scalar_mul(Qt[c][:], ps[:], WINV)
        for th in range(2):
            sl = slice(th * 512, (th + 1) * 512)
            ps = pj_psum.tile([P, 512], F32, name=f"psk_{c}_{th}", tag="proj", bufs=2)
            for kp in range(KCP):
                nc.tensor.matmul(ps[:], wk1t[c][:, kp], ln18[kp][:, :, sl],
                                 start=(kp == 0), stop=(kp == KCP - 1), perf_mode=DR)
            nc.vector.tensor_scalar_mul(Kt[c][:, sl], ps[:], WINV)

    def vproj_filler(nt):
        n0, nsz = ((0, 512), (512, 512), (1024, 256))[nt]
        if nt == 0:
            for j in range(4):
                nc.gpsimd.memset(V8[j][:], 1.0)
        for t8 in range(8):
            ps = pj_psum.tile([P, 512], F32, name=f"psv_{t8}_{n0}", tag="proj", bufs=2)
            for kp in range(KCP):
                nc.tensor.matmul(ps[:, :nsz], ln18[kp][:, :, t8 * P:(t8 + 1) * P],
                                 wv1t[kp][:, :, n0:n0 + nsz],
                                 start=(kp == 0), stop=(kp == KCP - 1), perf_mode=DR)
            nc.vector.tensor_scalar_mul(
                V8[t8 // 2][:, t8 % 2, n0 // DHEAD:(n0 + nsz) // DHEAD, 0:DHEAD],
                ps[:, :nsz].rearrange("p (h e) -> p h e", e=DHEAD), WINV)

    def attnv_dr(pc, exps, dov):
        for j in range(4):
            for h in range(2):
                nc.tensor.matmul(dov[h][:], V8[j][:, :, 2 * pc + h, 0:DHEAD + 1],
                                 exps[j][:, :, h * 512:(h + 1) * 512],
                                 start=(j == 0), stop=(j == 3), perf_mode=DR)

    def finish1(pc, dov):
        attn_finish(2 * pc, dov[0], Ot8[pc // 2][0:DHEAD, pc % 2, :], pj_psum, "proj")
        attn_finish(2 * pc + 1, dov[1], Ot8[pc // 2][DHEAD:2 * DHEAD, pc % 2, :],
                    pj_psum, "proj")

    pend = []  # (pair_idx, [4 exp pair tiles])
    qk_proj(0)
    for c in range(KC):
        if c + 1 < KC:
            qk_proj(c + 1)
        drain = pend.pop(0) if len(pend) >= 1 else None
        if drain is not None:
            dov = [ov_psum.tile([DHEAD + 1, 512], F32, name=f"ov{2 * drain[0] + h}",
                                tag="ov", bufs=2) for h in range(2)]
        exps = []
        for k8 in range(8):
            sps = sc_psum.tile([P, 1024], F32, name=f"sps{c}_{k8}", tag="sc", bufs=2)
            for h in range(2):
                nc.tensor.matmul(sps[:, h * 512:(h + 1) * 512],
                                 Kt[c][64 * h:64 * h + 64, k8 * P:(k8 + 1) * P],
                                 Qt[c][64 * h:64 * h + 64, :],
                                 start=True, stop=True, tile_position=(64 * h, 0))
            if k8 % 2 == 0:
                e8 = (epoolA if c % 2 == 0 else epoolB).tile(
                    [P, 2, 1024], F8, name=f"exp{c}_{k8 // 2}", tag="exp")
                exps.append(e8)
            nc.scalar.activation(e8[:, k8 % 2, :], sps[:], AF.Exp, scale=SCALE)
            if drain is not None and k8 % 2 == 1:
                j = k8 // 2
                for h in range(2):
                    nc.tensor.matmul(dov[h][:],
                                     V8[j][:, :, 2 * drain[0] + h, 0:DHEAD + 1],
                                     drain[1][j][:, :, h * 512:(h + 1) * 512],
                                     start=(j == 0), stop=(j == 3), perf_mode=DR)
        if drain is not None:
            finish1(drain[0], dov)
        if c < 3:
            vproj_filler(c)
        pend.append((c, exps))
    while pend:
        pc, exps = pend.pop(0)
        dov = [ov_psum.tile([DHEAD + 1, 512], F32, name=f"ovt{2 * pc + h}",
                            tag="ov", bufs=2) for h in range(2)]
        attnv_dr(pc, exps, dov)
        finish1(pc, dov)

    ln1p.release()
    wqk.release()

    # K2/V2 from context (tiny; drains while the attn tail finishes)
    for mc in range(KC):
        ps = pj_psum.tile([P, 512], F32, name=f"psk2_{mc}", tag="proj", bufs=2)
        for kp in range(KCXP):
            nc.tensor.matmul(ps[:, 0:MCTX], wk2t[mc][:, kp], ctx8[kp][:, :, 0:MCTX],
                             start=(kp == 0), stop=(kp == KCXP - 1), perf_mode=DR)
        nc.vector.tensor_scalar_mul(K2t[mc][:], ps[:, 0:MCTX], WINV)
    nc.gpsimd.memset(V2t[:], 1.0)
    for n0, nsz in ((0, 512), (512, 512), (1024, 256)):
        ps = pj_psum.tile([P, 512], F32, name=f"psv2_{n0}", tag="proj", bufs=2)
        for kp in range(KCXP):
            nc.tensor.matmul(ps[0:MCTX, :nsz], ctx8[kp][:, :, 0:MCTX],
                             wv2s[kp][:, :, n0:n0 + nsz],
                             start=(kp == 0), stop=(kp == KCXP - 1), perf_mode=DR)
        nc.vector.tensor_scalar_mul(
            V2t[:MCTX, n0 // DHEAD:(n0 + nsz) // DHEAD, 0:DHEAD],
            ps[0:MCTX, :nsz].rearrange("p (h e) -> p h e", e=DHEAD), WINV)

    epoolB.release()
    epoolA.release()
    v8p.release()
    qkt.release()
    pj_psum.release()
    ov_psum.release()
    sc_psum.release()

    # ============ phase B: O1 + LN2 + Q2 + attn2 + O2 + LN3 ============

    wffp = tc.alloc_tile_pool(name="wffp", bufs=1, side="right")
    wff1g, wff1a = [], []
    for j in range(JFF):
        wg = wffp.tile([P, KC, P], F16, name=f"wg_{j}", tag="wff1g", bufs=8)
        nc.sync.dma_start(wg[:], d["wff1"][JFF + j])
        wff1g.append(wg)
        wa = wffp.tile([P, KC, P], F16, name=f"wa_{j}", tag="wff1a", bufs=8)
        nc.sync.dma_start(wa[:], d["wff1"][j])
        wff1a.append(wa)

    psB1 = tc.alloc_tile_pool(name="psB1", bufs=1, space="PSUM")

    def proj8_ln(psB, wt_tiles, rhs_pairs, n_kp, consume, ln_idx, x_out, res_tiles,
                 bias0):
        """x_out[mc] = psum/WS (+bias) + res; LN stats inlined; finalize."""
        sums_ps = psB.tile([1, 512], F32, name=f"lns{ln_idx}", tag="lnstat", bufs=2)
        sq_ps = psB.tile([1, 512], F32, name=f"lnq{ln_idx}", tag="lnstat", bufs=2)
        for mc in range(KC):
            ps = psB.tile([P, 512], F32, name=f"ps{ln_idx}_{mc}", tag="proj", bufs=2)
            for kp in range(n_kp):
                nc.tensor.matmul(ps[:], wt_tiles[mc][:, kp], rhs_pairs(kp),
                                 start=(kp == 0), stop=(kp == n_kp - 1), perf_mode=DR)
            if trivial_bias:
                nc.vector.scalar_tensor_tensor(x_out[mc][:], ps[:], WINV,
                                               res_tiles[mc][:], MUL, ADD)
            else:
                xb = tmp.tile([P, T], F32, name=f"xb{ln_idx}_{mc}", tag="xb", bufs=2)
                nc.scalar.activation(xb[:], ps[:], AF.Copy, scale=WINV,
                                     bias=bias_ap(bias0 + mc))
                nc.vector.tensor_add(x_out[mc][:], xb[:], res_tiles[mc][:])
            xsq = tmp.tile([P, T], F16, name=f"xsqB{ln_idx}_{mc}", tag="xsq", bufs=3)
            (nc.vector if mc % 2 == 0 else nc.gpsimd).tensor_mul(
                xsq[:], x_out[mc][:], x_out[mc][:])
            ln_stats_mm(x_out[mc][:], xsq, sums_ps, sq_ps, mc == 0, mc == KC - 1)

        def out(c, mu16, rstd16):
            ln_out(ln_idx, 0, c, x_out[c][:], mu16, rstd16, consume(c),
                   eng=(nc.vector if c % 2 == 0 else nc.gpsimd))

        ln_finalize(ln_idx, 0, psB, sums_ps, sq_ps, out)

    # x1 is written in place into the residual tiles.
    x1 = resid
    proj8_ln(psB1, wo1t, lambda kp: Ot8[kp][:], KCP,
             lambda c: ln28[c // 2][:, c % 2, :], 1, x1, resid, 0)
    otp.release()
    wpre.release()

    # Q2 projection
    for mc in range(KC):
        ps = psB1.tile([P, 512], F32, name=f"psq2_{mc}", tag="proj", bufs=2)
        for kp in range(KCP):
            nc.tensor.matmul(ps[:], wq2t[mc][:, kp], ln28[kp][:, :, :],
                             start=(kp == 0), stop=(kp == KCP - 1), perf_mode=DR)
        nc.vector.tensor_scalar_mul(Q2t[mc][:], ps[:], WINV)
    ln2p.release()
    wq2p.release()
    psB1.release()

    # attn2: 77 context keys, fp16, single contraction chunk
    psA2 = tc.alloc_tile_pool(name="psA2", bufs=1, space="PSUM")
    e2pool = tc.alloc_tile_pool(name="epool2", bufs=6, side="right")

    def attn2_drain(dc, de):
        dov = [psA2.tile([DHEAD + 1, 512], F32, name=f"ov2_{2 * dc + h}",
                         tag="ov2", bufs=4) for h in range(2)]
        for h in range(2):
            nc.tensor.matmul(dov[h][:], V2t[:MCTX, 2 * dc + h, :],
                             de[:, h * 512:(h + 1) * 512], start=True, stop=True)
        attn_finish(2 * dc, dov[0], O2t8[dc // 2][0:DHEAD, dc % 2, :], psA2, "ov2",
                    ps_bufs=4)
        attn_finish(2 * dc + 1, dov[1], O2t8[dc // 2][DHEAD:2 * DHEAD, dc % 2, :],
                    psA2, "ov2", ps_bufs=4)

    pend2 = []
    for c in range(KC):
        if len(pend2) >= 2:
            attn2_drain(*pend2.pop(0))
        sps = psA2.tile([MCTX, 1024], F32, name=f"sps2_{c}", tag="sc2", bufs=2)
        for h in range(2):
            nc.tensor.matmul(sps[:, h * 512:(h + 1) * 512],
                             K2t[c][64 * h:64 * h + 64, 0:MCTX],
                             Q2t[c][64 * h:64 * h + 64, :],
                             start=True, stop=True, tile_position=(64 * h, 0))
        e = e2pool.tile([MCTX, 1024], F16, name=f"exp2_{c}", tag="exp2")
        nc.scalar.activation(e[:], sps[:], AF.Exp, scale=SCALE)
        pend2.append((c, e))
    while pend2:
        attn2_drain(*pend2.pop(0))
    e2pool.release()
    psA2.release()
    q2p.release()
    k2v2.release()

    # O2 + residual + LN3 (fp16 out feeding the fp16 GEGLU up-proj)
    wo2p = tc.alloc_tile_pool(name="wo2p", bufs=1)       # * (leaked)
    wo2t = []
    for mc in range(KC):
        wt = wo2p.tile([P, KCP, 2, P], F8, name=f"wo2_{mc}", tag="wo2", bufs=KC)
        nc.sync.dma_start(wt[:], d["wo2"][mc])
        wo2t.append(wt)
    x2p = tc.alloc_tile_pool(name="x2p", bufs=1)
    x2 = [x2p.tile([P, T], F16, name=f"x2_{mc}", tag="x2", bufs=KC) for mc in range(KC)]
    ln3p = tc.alloc_tile_pool(name="ln3p", bufs=1)       # * (leaked)
    ln3t = [ln3p.tile([P, T], F16, name=f"ln3_{c}", tag="ln3", bufs=KC)
            for c in range(KC)]
    psB2 = tc.alloc_tile_pool(name="psB2", bufs=1, space="PSUM")
    proj8_ln(psB2, wo2t, lambda kp: O2t8[kp][:], KCP,
             lambda c: ln3t[c][:], 2, x2, x1, 10)
    psB2.release()

    # ============ phase C: GEGLU up-proj (fp16) ============

    hhp = tc.alloc_tile_pool(name="hhp", bufs=1)
    hh8 = [hhp.tile([P, 2, T], F8, name=f"hh8_{i}", tag="hh8", bufs=JFP)
           for i in range(JFP)]

    proj_psum = tc.alloc_tile_pool(name="proj_psum4", bufs=1, space="PSUM")
    for j in range(JFF):
        gps = proj_psum.tile([P, 512], F32, name=f"gps_{j}", tag="proj", bufs=4)
        for kc in range(KC):
            nc.tensor.matmul(gps[:], wff1g[j][:, kc], ln3t[kc][:], start=(kc == 0),
                             stop=(kc == KC - 1))
        gel = tmp.tile([P, T], F16, name=f"gel_{j}", tag="gel", bufs=3)
        if trivial_bias:
            nc.scalar.activation(gel[:], gps[:], AF.Gelu_apprx_tanh)
        else:
            nc.scalar.activation(gel[:], gps[:], AF.Gelu_apprx_tanh,
                                 bias=bias_ap(60 + j))
        aps = proj_psum.tile([P, 512], F32, name=f"aps_{j}", tag="proj", bufs=4)
        for kc in range(KC):
            nc.tensor.matmul(aps[:], wff1a[j][:, kc], ln3t[kc][:], start=(kc == 0),
                             stop=(kc == KC - 1))
        if trivial_bias:
            nc.vector.tensor_mul(hh8[j // 2][:, j % 2, :], aps[:], gel[:])
        else:
            nc.vector.scalar_tensor_tensor(hh8[j // 2][:, j % 2, :], aps[:],
                                           bias_ap(20 + j), gel[:], ADD, MUL)
    wffp.release()

    # ============ phase D: FF down-proj (fp8 DoubleRow) + residual ============

    wf2p = tc.alloc_tile_pool(name="wf2p", bufs=1)
    outp = tc.alloc_tile_pool(name="outp", bufs=4)
    for mc in range(KC):
        wt = wf2p.tile([P, JFP, 2, P], F8, name=f"wff2_{mc}", tag="wff2", bufs=2)
        nc.sync.dma_start(wt[:], d["wff2"][mc])
        ps = proj_psum.tile([P, 512], F32, name=f"psf2_{mc}", tag="proj", bufs=4)
        for kp in range(JFP):
            nc.tensor.matmul(ps[:], wt[:, kp], hh8[kp][:], start=(kp == 0),
                             stop=(kp == JFP - 1), perf_mode=DR)
        ot = outp.tile([P, T], F32, name=f"out_{mc}", tag="out")
        if trivial_bias:
            nc.vector.scalar_tensor_tensor(ot[:], ps[:], WINV, x2[mc][:], MUL, ADD)
        else:
            xb = tmp.tile([P, T], F32, name=f"xbf2_{mc}", tag="xb", bufs=2)
            nc.scalar.activation(xb[:], ps[:], AF.Copy, scale=WINV,
                                 bias=bias_ap(100 + mc))
            nc.vector.tensor_add(ot[:], xb[:], x2[mc][:])
        nc.sync.dma_start(d["out"][mc * P:(mc + 1) * P, :], ot[:])

    outp.release()
    wf2p.release()
    proj_psum.release()
    hhp.release()
    ln3p.release()
    x2p.release()
    wo2p.release()
    o2p.release()
    resp.release()
    ctxp.release()
    tmp.release()
    const.release()


def _q8(w, scale=WS):
    return np.clip(np.asarray(w, np.float32) * scale, -240.0, 240.0).astype(
        ml_dtypes.float8_e4m3)


def _lhst8_layout(w, n_kc, n_mc):
    """[K, M] f32 -> fp8 [n_mc, 128, n_kc/2, 2, 128]: block [mc] is the
    DoubleRow stationary group for output chunk mc (dim -2 pairs two adjacent
    contraction chunks)."""
    a = w.reshape(n_kc // 2, 2, P, n_mc, P).transpose(3, 2, 0, 1, 4)
    return np.ascontiguousarray(_q8(a))


def _rhs8_layout(w, n_kc):
    """[K, M] f32 -> fp8 [n_kc/2, 128, 2, M] moving-operand pair layout."""
    a = w.reshape(n_kc // 2, 2, P, -1).transpose(0, 2, 1, 3)
    return np.ascontiguousarray(_q8(a))


def _lhst_layout(w, n_kc, n_mc):
    """[K, M] f32 -> fp16 [n_mc, 128, n_kc, 128] plain stationary groups."""
    return np.ascontiguousarray(
        w.reshape(n_kc, P, n_mc, P).transpose(2, 1, 0, 3).astype(np.float16))


def _ctx8_layout(ctx):
    """[MCTX, CTX_DIM] f32 -> fp8 [KCXP, 128, 2, MPAD] feature-pair layout."""
    a = np.zeros((KCXP, P, 2, MPAD), np.float32)
    a[:, :, :, :MCTX] = ctx.T.reshape(KCXP, 2, P, MCTX).transpose(0, 2, 1, 3)
    return _q8(a, 1.0)


_BUILT = {}


def _build(trivial_aff, trivial_bias):
    key = (trivial_aff, trivial_bias)
    if key in _BUILT:
        return _BUILT[key]
    nc = bacc.Bacc("TRN2", target_bir_lowering=False, debug=False, num_devices=N_CORES)
    d = {
        "xt": nc.dram_tensor("xt", [DIM, TKV], F16, kind="ExternalInput").ap(),
        "ctx8": nc.dram_tensor("ctx8", [KCXP, P, 2, MPAD], F8, kind="ExternalInput").ap(),
        "wq1": nc.dram_tensor("wq1", [KC, P, KCP, 2, P], F8, kind="ExternalInput").ap(),
        "wk1": nc.dram_tensor("wk1", [KC, P, KCP, 2, P], F8, kind="ExternalInput").ap(),
        "wv1": nc.dram_tensor("wv1", [KCP, P, 2, DIM], F8, kind="ExternalInput").ap(),
        "wo1": nc.dram_tensor("wo1", [KC, P, KCP, 2, P], F8, kind="ExternalInput").ap(),
        "wq2": nc.dram_tensor("wq2", [KC, P, KCP, 2, P], F8, kind="ExternalInput").ap(),
        "wk2": nc.dram_tensor("wk2", [KC, P, KCXP, 2, P], F8, kind="ExternalInput").ap(),
        "wv2": nc.dram_tensor("wv2", [KCXP, P, 2, DIM], F8, kind="ExternalInput").ap(),
        "wo2": nc.dram_tensor("wo2", [KC, P, KCP, 2, P], F8, kind="ExternalInput").ap(),
        "wff1": nc.dram_tensor("wff1", [2 * JFF, P, KC, P], F16, kind="ExternalInput").ap(),
        "wff2": nc.dram_tensor("wff2", [KC, P, JFP, 2, P], F8, kind="ExternalInput").ap(),
        "out": nc.dram_tensor("out", [DIM, T], F32, kind="ExternalOutput").ap(),
    }
    if not trivial_aff:
        d["aff"] = nc.dram_tensor("aff", [P, 60], F32, kind="ExternalInput").ap()
    if not trivial_bias:
        d["biases"] = nc.dram_tensor("biases", [P, 110], F32, kind="ExternalInput").ap()
    with tile.TileContext(nc, pool_alloc_mode="queue") as tc:
        _emit(tc, d, trivial_aff, trivial_bias)
    nc.compile()
    _BUILT[key] = nc
    return nc


def kernel(x, context,
           g1, be1, wq1, wk1, wv1, wo1, bo1,
           g2, be2, wq2, wk2, wv2, wo2, bo2,
           g3, be3, w_ff1, b_ff1, w_ff2, b_ff2,
           _trace=False):
    global last_exec_time_ns
    x = np.asarray(x, np.float32)
    context = np.asarray(context, np.float32)

    affs = [np.asarray(a, np.float32) for a in (g1, be1, g2, be2, g3, be3)]
    biases = [np.asarray(b, np.float32) for b in (bo1, bo2, b_ff1, b_ff2)]
    trivial_aff = all(np.all(a == (1.0 if i % 2 == 0 else 0.0))
                      for i, a in enumerate(affs))
    trivial_bias = all(np.all(b == 0.0) for b in biases)

    nc = _build(trivial_aff, trivial_bias)

    shared = {
        "wq1": _lhst8_layout(np.asarray(wq1, np.float32), KC, KC),
        "wk1": _lhst8_layout(np.asarray(wk1, np.float32), KC, KC),
        "wv1": _rhs8_layout(np.asarray(wv1, np.float32), KC),
        "wo1": _lhst8_layout(np.asarray(wo1, np.float32), KC, KC),
        "wq2": _lhst8_layout(np.asarray(wq2, np.float32), KC, KC),
        "wk2": _lhst8_layout(np.asarray(wk2, np.float32), KCX, KC),
        "wv2": _rhs8_layout(np.asarray(wv2, np.float32), KCX),
        "wo2": _lhst8_layout(np.asarray(wo2, np.float32), KC, KC),
        "wff1": _lhst_layout(np.asarray(w_ff1, np.float32), KC, 2 * JFF),
        "wff2": _lhst8_layout(np.asarray(w_ff2, np.float32), JFF, KC),
    }
    if not trivial_aff:
        aff = np.zeros([P, 60], np.float32)
        for i, a in enumerate(affs):
            # col = ln_idx*20 + (0 for g / 10 for be) + chunk
            ln_idx, j = i // 2, i % 2
            aff[:, ln_idx * 20 + j * 10: ln_idx * 20 + j * 10 + 10] = \
                a.reshape(KC, P).T
        shared["aff"] = aff
    if not trivial_bias:
        bb = np.zeros([P, 110], np.float32)
        bb[:, 0:10] = biases[0].reshape(KC, P).T
        bb[:, 10:20] = biases[1].reshape(KC, P).T
        bb[:, 20:100] = biases[2].reshape(2 * JFF, P).T
        bb[:, 100:110] = biases[3].reshape(KC, P).T
        shared["biases"] = bb

    in_maps = []
    for b in range(BATCH):
        ctx8 = _ctx8_layout(context[b])
        for h in range(2):
            xr = np.roll(x[b], -h * T, axis=0)
            m = dict(shared)
            m["xt"] = np.ascontiguousarray(xr.T.astype(np.float16))
            m["ctx8"] = ctx8
            in_maps.append(m)

    res = bass_utils.run_bass_kernel_spmd(
        nc, in_maps, core_ids=list(range(N_CORES)), trace=_trace)
    last_exec_time_ns = res.exec_time_ns

    out = np.empty((BATCH, NTOK, DIM), np.float32)
    for b in range(BATCH):
        for h in range(2):
            out[b, h * T:(h + 1) * T, :] = res.results[b * 2 + h]["out"].T
    return out


# revision 54
# speedup vs baseline: 1.1247x; 1.1247x over previous
"""BasicTransformerBlock on 8 TRN2 NeuronCores.

Sharding: data-parallel, core = (batch b in 0..3) x (sequence half h in 0..1).
Each core receives its batch element's full sequence rotated so its local 512
rows come first (softmax over keys is permutation invariant), computes K/V of
attn1 for all 1024 tokens (duplicated across the pair, ~10% extra FLOPs, zero
collectives), and everything else for its 512 local tokens only.

On-chip layout: feature-major activations [features on partitions, tokens on
free axis]. The residual stream, LN math and PSUM accumulation stay fp32.
LayerNorm partition reductions and per-token broadcasts use fp16 ones-matmuls.
Attention softmax denominators come free from a ones-column appended to V.

Precision: Q/K/V/O projections (both attns), attn1's attnV and the FF
down-proj run in fp8 e4m3 with DoubleRow matmuls (two 128-row contraction
chunks per pass, 2x PE rate); weights are pre-scaled x32 on the host and the
PSUM rescaled by 1/32 at evacuation. The GEGLU up-proj (the dominant error
path) stays fp16. Attention scores use fp8 operands at the normal rate.

Schedule: Q/K projections run one head-pair ahead of the score matmuls inside
a single fused LN1+proj+attention phase, so the PE streams continuously while
ACT chases with exp. LN2/LN3 statistics are inlined into the O-projection
consume loops. K2/V2 run at the attn1 tail; FF weights stream in early on
wide rolling pools. SBUF pools use the queue (ring) allocator so each pool
releases as soon as its contents die.
"""

import sys
import types

sys.path.insert(0, "/opt/trn_rl_repo")

# concourse fetches the NTFF profile hook from antenv.axon_hooks, which the
# agent image's antenv stub lacks. Register a shim so trace=True works.
if "antenv.axon_hooks" not in sys.modules:
    _hooks = types.ModuleType("antenv.axon_hooks")
    _HOOK = [None]

    def _get_hook():
        if _HOOK[0] is None:
            try:
                from trn_agent_boot.trn_boot import _ntff_profile_via_ctypes

                _HOOK[0] = _ntff_profile_via_ctypes("/opt/axon/libaxon_pjrt.so")
            except Exception:
                _HOOK[0] = None
        return _HOOK[0]

    _hooks.get_axon_ntff_profile_hook = _get_hook
    _hooks.set_axon_ntff_profile_hook = lambda h: _HOOK.__setitem__(0, h)
    sys.modules["antenv.axon_hooks"] = _hooks
    try:
        import antenv

        antenv.axon_hooks = _hooks
    except ImportError:
        pass

import ml_dtypes
import numpy as np

import concourse.bass as bass
import concourse.mybir as mybir
import concourse.tile as tile
from concourse import bacc, bass_utils

dt = mybir.dt
F32, F16, F8 = dt.float32, dt.float16, dt.float8e4
AF = mybir.ActivationFunctionType
DR = mybir.MatmulPerfMode.DoubleRow
MUL, ADD, SUB = mybir.AluOpType.mult, mybir.AluOpType.add, mybir.AluOpType.subtract

DIM, HEADS, DHEAD, CTX_DIM, DFF = 1280, 20, 64, 768, 5120
BATCH, NTOK, MCTX = 4, 1024, 77
EPS = 1e-5
SCALE = DHEAD ** -0.5
N_CORES = 8
T = 512         # local tokens per core
TKV = 1024      # attn1 key/value tokens per core
KC = DIM // 128           # 10
KCP = KC // 2             # 5 contraction pairs
KCX = CTX_DIM // 128      # 6
KCXP = KCX // 2           # 3
JFF = DFF // 128          # 40 (chunks of the gated hidden)
JFP = JFF // 2            # 20 pairs for the down-proj contraction
P = 128
WS = 32.0                 # fp8 weight pre-scale (power of two)
WINV = 1.0 / WS
VPAD = 68                 # DHEAD+1 padded so the V pair stride is 16B aligned
MPAD = 80                 # MCTX padded likewise for the ctx pair stride

last_exec_time_ns = None


def _emit(tc, d, trivial_aff, trivial_bias):
    nc = tc.nc

    const = tc.alloc_tile_pool(name="const", bufs=1)
    ones_col = const.tile([P, 1], F16, name="ones_col")
    nc.vector.memset(ones_col[:], 1.0)
    ones_row = const.tile([1, P], F16, name="ones_row")
    nc.vector.memset(ones_row[:], 1.0)
    if not trivial_aff:
        aff = const.tile([P, 60], F32, name="aff")
        nc.sync.dma_start(aff[:], d["aff"])
    if not trivial_bias:
        biases = const.tile([P, 110], F32, name="biases")
        nc.sync.dma_start(biases[:], d["biases"])

    tmp = tc.alloc_tile_pool(name="tmp", bufs=1)

    # ---------------- helpers ----------------

    def bias_ap(col):
        return biases[:, col:col + 1]

    def ln_stats_mm(xh_ap, xsq, sums_ps, sq_ps, start, stop):
        nc.tensor.matmul(sums_ps[:], ones_col[:], xh_ap, start=start, stop=stop)
        nc.tensor.matmul(sq_ps[:], ones_col[:], xsq[:], start=start, stop=stop)

    def ln_finalize(ln_idx, t, psum_p, sums_ps, sq_ps, out_fn):
        """Turn accumulated sum / sum-sq PSUMs into mu/rstd and emit the
        normalized outputs via out_fn(c, mu16, rstd16)."""
        ssum = tmp.tile([1, 512], F16, name=f"ssum{ln_idx}_{t}", tag="ssum", bufs=2)
        nc.vector.tensor_copy(out=ssum[:], in_=sums_ps[:])
        ssq = tmp.tile([1, 512], F16, name=f"ssq{ln_idx}_{t}", tag="ssq", bufs=2)
        nc.vector.tensor_copy(out=ssq[:], in_=sq_ps[:])
        bs_ps = psum_p.tile([P, 512], F32, name=f"bs{ln_idx}_{t}", tag="proj", bufs=2)
        nc.tensor.matmul(bs_ps[:], ones_row[:], ssum[:], start=True, stop=True)
        bq_ps = psum_p.tile([P, 512], F32, name=f"bq{ln_idx}_{t}", tag="proj", bufs=2)
        nc.tensor.matmul(bq_ps[:], ones_row[:], ssq[:], start=True, stop=True)
        mu16 = tmp.tile([P, 512], F16, name=f"mu16_{ln_idx}_{t}", tag="mu16", bufs=2)
        nc.vector.tensor_scalar_mul(mu16[:], bs_ps[:], 1.0 / DIM)
        musq = tmp.tile([P, 512], F16, name=f"musq{ln_idx}_{t}", tag="musq", bufs=1)
        nc.vector.tensor_mul(musq[:], mu16[:], mu16[:])
        # musq - EPS, so var = ex2 - musq + EPS below
        nc.vector.tensor_scalar_sub(musq[:], musq[:], EPS)
        var = tmp.tile([P, 512], F16, name=f"var{ln_idx}_{t}", tag="var", bufs=1)
        nc.vector.scalar_tensor_tensor(var[:], bq_ps[:], 1.0 / DIM, musq[:], MUL, SUB)
        std = tmp.tile([P, 512], F32, name=f"std{ln_idx}_{t}", tag="std", bufs=1)
        nc.scalar.sqrt(std[:], var[:])
        rstd = tmp.tile([P, 512], F32, name=f"rstd{ln_idx}_{t}", tag="rstd", bufs=1)
        nc.vector.reciprocal_approx_fast(rstd[:], std[:])
        rstd16 = tmp.tile([P, 512], F16, name=f"rstd16_{ln_idx}_{t}", tag="rstd16",
                          bufs=2)
        nc.vector.tensor_copy(out=rstd16[:], in_=rstd[:])
        for c in range(KC):
            out_fn(c, mu16, rstd16)

    def ln_out(ln_idx, t, c, src_ap, mu16, rstd16, out_ap, eng=None):
        """out = (src - mu) * rstd (* g + b)."""
        eng = eng or nc.vector
        xm = tmp.tile([P, 512], F16, name=f"xm{ln_idx}_{t}_{c}", tag="xm", bufs=4)
        eng.tensor_sub(xm[:], src_ap, mu16[:])
        if trivial_aff:
            eng.tensor_mul(out_ap, xm[:], rstd16[:])
        else:
            xn = tmp.tile([P, 512], F16, name=f"xn{ln_idx}_{t}_{c}", tag="xn", bufs=3)
            nc.vector.tensor_mul(xn[:], xm[:], rstd16[:])
            g_ap = aff[:, ln_idx * 20 + c: ln_idx * 20 + c + 1]
            be_ap = aff[:, ln_idx * 20 + 10 + c: ln_idx * 20 + 10 + c + 1]
            xg = tmp.tile([P, 512], F16, name=f"xg{ln_idx}_{t}_{c}", tag="xg", bufs=3)
            nc.vector.tensor_scalar_mul(xg[:], xn[:], g_ap)
            nc.scalar.activation(out_ap, xg[:], AF.Copy, bias=be_ap)

    def attn_finish(head, ops_, out_ap, psum_p, ps_tag, evac_act=False, ps_bufs=2):
        usb = tmp.tile([DHEAD + 1, 512], F16, name=f"usb{head}", tag="usb", bufs=2)
        if evac_act:
            nc.scalar.copy(usb[:], ops_[:])
        else:
            nc.vector.tensor_copy(out=usb[:], in_=ops_[:])
        den = tmp.tile([1, 512], F32, name=f"den{head}", tag="den", bufs=2)
        if evac_act:
            nc.scalar.copy(den[:], ops_[DHEAD:DHEAD + 1, :])
        else:
            nc.vector.tensor_copy(out=den[:], in_=ops_[DHEAD:DHEAD + 1, :])
        rec32 = tmp.tile([1, 512], F32, name=f"rec32_{head}", tag="rec32", bufs=2)
        nc.vector.reciprocal_approx_fast(rec32[:], den[:])
        rec = tmp.tile([1, 512], F16, name=f"rec{head}", tag="rec", bufs=2)
        if evac_act:
            nc.scalar.copy(rec[:], rec32[:])
        else:
            nc.vector.tensor_copy(out=rec[:], in_=rec32[:])
        bps = psum_p.tile([DHEAD, 512], F32, name=f"bps{head}", tag=ps_tag,
                          bufs=ps_bufs)
        nc.tensor.matmul(bps[:], ones_row[:, :DHEAD], rec[:], start=True, stop=True)
        nc.vector.tensor_mul(out_ap, usb[:DHEAD, :], bps[:])

    # ============ phase A: LN1 + QKV projections + attn1, fused ============
    # Left-side pools open in death-descending order (LIFO release discipline;
    # pools marked * stay open to the end: the queue allocator makes the dead
    # space explicit and cheap).

    # x DMA is the critical path to LN1 — allocate xpool first and emit its
    # DMAs before any weight DMA so the DMA queues serve it first.
    xpool = tc.alloc_tile_pool(name="xpool", bufs=1, side="right")
    x_sb = []
    for c in range(KC):
        xc = xpool.tile([P, TKV], F16, name=f"x_{c}", tag="x", bufs=KC)
        nc.sync.dma_start(xc[:], d["xt"][c * P:(c + 1) * P, :])
        x_sb.append(xc)

    ctxp = tc.alloc_tile_pool(name="ctxp", bufs=1)          # * (tiny)
    ctx8 = []
    for kp in range(KCXP):
        c8 = ctxp.tile([P, 2, MPAD], F8, name=f"ctx8_{kp}", tag="ctx8", bufs=KCXP)
        nc.sync.dma_start(c8[:], d["ctx8"][kp])
        ctx8.append(c8)

    # Residual slice of x; O1 writes x1 into these tiles in place.      *
    resp = tc.alloc_tile_pool(name="resp", bufs=1)
    resid = [resp.tile([P, T], F16, name=f"res_{c}", tag="res", bufs=KC)
             for c in range(KC)]

    o2p = tc.alloc_tile_pool(name="o2p", bufs=1)            # * (attn2 out)
    O2t8 = [o2p.tile([P, 2, T], F8, name=f"o2t8_{i}", tag="o2t8", bufs=KCP)
            for i in range(KCP)]

    k2v2 = tc.alloc_tile_pool(name="k2v2", bufs=1)          # dies at attn2
    K2t = [k2v2.tile([P, MCTX], F16, name=f"k2t_{mc}", tag="k2t", bufs=KC)
           for mc in range(KC)]
    V2t = k2v2.tile([P, HEADS, DHEAD + 1], F16, name="v2t", tag="v2t", bufs=1)

    q2p = tc.alloc_tile_pool(name="q2p", bufs=1)            # dies at attn2
    Q2t = [q2p.tile([P, T], F16, name=f"q2t_{mc}", tag="q2t", bufs=KC)
           for mc in range(KC)]

    wq2p = tc.alloc_tile_pool(name="wq2p", bufs=1)          # dies after Q2
    wq2t = [wq2p.tile([P, KCP, 2, P], F8, name=f"wq2_{mc}", tag="wq2", bufs=KC)
            for mc in range(KC)]

    ln2p = tc.alloc_tile_pool(name="ln2p", bufs=1)          # dies after Q2
    ln28 = [ln2p.tile([P, 2, T], F8, name=f"ln28_{i}", tag="ln28", bufs=KCP)
            for i in range(KCP)]

    wpre = tc.alloc_tile_pool(name="wpre", bufs=1)          # dies after O1
    wo1t = [wpre.tile([P, KCP, 2, P], F8, name=f"wo1_{mc}", tag="wo1", bufs=KC)
            for mc in range(KC)]
    wk2t = [wpre.tile([P, KCXP, 2, P], F8, name=f"wk2_{mc}", tag="wk2", bufs=KC)
            for mc in range(KC)]
    wv2s = [wpre.tile([P, 2, DIM], F8, name=f"wv2_{kp}", tag="wv2", bufs=KCXP)
            for kp in range(KCXP)]

    otp = tc.alloc_tile_pool(name="otp", bufs=1)            # dies after O1
    Ot8 = [otp.tile([P, 2, T], F8, name=f"ot8_{i}", tag="ot8", bufs=KCP)
           for i in range(KCP)]

    wqk = tc.alloc_tile_pool(name="wqk", bufs=1)            # dies at pipe end
    wq1t, wk1t, wv1t = [], [], []
    for mc in range(KC):
        wq1t.append(wqk.tile([P, KCP, 2, P], F8, name=f"wq1_{mc}", tag="wq1", bufs=KC))
        wk1t.append(wqk.tile([P, KCP, 2, P], F8, name=f"wk1_{mc}", tag="wk1", bufs=KC))
    wv1t = [wqk.tile([P, 2, DIM], F8, name=f"wv1_{kp}", tag="wv1", bufs=KCP)
            for kp in range(KCP)]

    ln1p = tc.alloc_tile_pool(name="ln1p", bufs=1)          # dies at pipe end
    ln18 = [ln1p.tile([P, 2, TKV], F8, name=f"ln18_{i}", tag="ln18", bufs=KCP)
            for i in range(KCP)]

    # Weight DMAs, in order of first use (after x so they can't delay LN1).
    for mc in range(KC):
        nc.sync.dma_start(wq1t[mc][:], d["wq1"][mc])
        nc.sync.dma_start(wk1t[mc][:], d["wk1"][mc])
    for kp in range(KCP):
        nc.sync.dma_start(wv1t[kp][:], d["wv1"][kp])
    for c in range(KC):
        nc.sync.dma_start(resid[c][:], d["xt"][c * P:(c + 1) * P, 0:T])
    for mc in range(KC):
        nc.sync.dma_start(wk2t[mc][:], d["wk2"][mc])
    for kp in range(KCXP):
        nc.sync.dma_start(wv2s[kp][:], d["wv2"][kp])
    for mc in range(KC):
        nc.sync.dma_start(wo1t[mc][:], d["wo1"][mc])
    for mc in range(KC):
        nc.sync.dma_start(wq2t[mc][:], d["wq2"][mc])

    # LN1 over all 1024 keys. x arrives fp16 and feeds the stats matmuls
    # directly; both 512-token tiles' stats run back-to-back on the PE, with
    # the normalize work split across DVE and GpSimd.
    ln_psum = tc.alloc_tile_pool(name="ln_psum", bufs=1, space="PSUM")
    stats = []
    for t in range(2):
        sl = slice(t * 512, (t + 1) * 512)
        sums_ps = ln_psum.tile([1, 512], F32, name=f"lns0_{t}", tag="lnstat", bufs=4)
        sq_ps = ln_psum.tile([1, 512], F32, name=f"lnq0_{t}", tag="lnstat", bufs=4)
        for c in range(KC):
            xsq = tmp.tile([P, 512], F16, name=f"xsq0_{t}_{c}", tag="xsq", bufs=3)
            if c % 2 == 0:
                nc.vector.tensor_mul(xsq[:], x_sb[c][:, sl], x_sb[c][:, sl])
            else:
                nc.scalar.activation(xsq[:], x_sb[c][:, sl], AF.Square)
            ln_stats_mm(x_sb[c][:, sl], xsq, sums_ps, sq_ps, c == 0, c == KC - 1)
        stats.append((sums_ps, sq_ps))
    for t in range(2):
        sl = slice(t * 512, (t + 1) * 512)

        def ln1_out(c, mu16, rstd16, t=t, sl=sl):
            ln_out(0, t, c, x_sb[c][:, sl], mu16, rstd16,
                   ln18[c // 2][:, c % 2, sl],
                   eng=(nc.gpsimd if c % 3 == 2 else nc.vector))

        ln_finalize(0, t, ln_psum, stats[t][0], stats[t][1], ln1_out)
    xpool.release()
    ln_psum.release()

    sc_psum = tc.alloc_tile_pool(name="sc_psum", bufs=1, space="PSUM")
    ov_psum = tc.alloc_tile_pool(name="ov_psum", bufs=1, space="PSUM")
    pj_psum = tc.alloc_tile_pool(name="pj_psum", bufs=1, space="PSUM")
    qkt = tc.alloc_tile_pool(name="qkt", bufs=1, side="right")
    v8p = tc.alloc_tile_pool(name="v8p", bufs=1, side="right")
    epoolA = tc.alloc_tile_pool(name="epoolA", bufs=4, side="right")
    epoolB = tc.alloc_tile_pool(name="epoolB", bufs=4, side="right")

    Qt = [qkt.tile([P, T], F8, name=f"qt_{mc}", tag="qt", bufs=KC) for mc in range(KC)]
    Kt = [qkt.tile([P, TKV], F8, name=f"kt_{mc}", tag="kt", bufs=KC) for mc in range(KC)]
    V8 = [v8p.tile([P, 2, HEADS, VPAD], F8, name=f"v8_{j}", tag="v8", bufs=4)
          for j in range(4)]

    def qk_proj(c):
        """Project Q chunk c (local tokens) and K chunk c (all 1024 keys)."""
        ps = pj_psum.tile([P, 512], F32, name=f"psq_{c}", tag="proj", bufs=2)
        for kp in range(KCP):
            nc.tensor.matmul(ps[:], wq1t[c][:, kp], ln18[kp][:, :, 0:T],
                             start=(kp == 0), stop=(kp == KCP - 1), perf_mode=DR)
        nc.vector.tensor_scalar_mul(Qt[c][:], ps[:], WINV)
        for th in range(2):
            sl = slice(th * 512, (th + 1) * 512)
            ps = pj_psum.tile([P, 512], F32, name=f"psk_{c}_{th}", tag="proj", bufs=2)
            for kp in range(KCP):
                nc.tensor.matmul(ps[:], wk1t[c][:, kp], ln18[kp][:, :, sl],
                                 start=(kp == 0), stop=(kp == KCP - 1), perf_mode=DR)
            nc.vector.tensor_scalar_mul(Kt[c][:, sl], ps[:], WINV)

    def vproj_filler(nt):
        n0, nsz = ((0, 512), (512, 512), (1024, 256))[nt]
        if nt == 0:
            for j in range(4):
                nc.gpsimd.memset(V8[j][:], 1.0)
        for t8 in range(8):
            ps = pj_psum.tile([P, 512], F32, name=f"psv_{t8}_{n0}", tag="proj", bufs=2)
            for kp in range(KCP):
                nc.tensor.matmul(ps[:, :nsz], ln18[kp][:, :, t8 * P:(t8 + 1) * P],
                                 wv1t[kp][:, :, n0:n0 + nsz],
                                 start=(kp == 0), stop=(kp == KCP - 1), perf_mode=DR)
            nc.vector.tensor_scalar_mul(
                V8[t8 // 2][:, t8 % 2, n0 // DHEAD:(n0 + nsz) // DHEAD, 0:DHEAD],
                ps[:, :nsz].rearrange("p (h e) -> p h e", e=DHEAD), WINV)

    def attnv_dr(pc, exps, dov):
        for j in range(4):
            for h in range(2):
                nc.tensor.matmul(dov[h][:], V8[j][:, :, 2 * pc + h, 0:DHEAD + 1],
                                 exps[j][:, :, h * 512:(h + 1) * 512],
                                 start=(j == 0), stop=(j == 3), perf_mode=DR)

    def finish1(pc, dov):
        attn_finish(2 * pc, dov[0], Ot8[pc // 2][0:DHEAD, pc % 2, :], ov_psum, "ov")
        attn_finish(2 * pc + 1, dov[1], Ot8[pc // 2][DHEAD:2 * DHEAD, pc % 2, :],
                    ov_psum, "ov")

    pend = []  # (pair_idx, [4 exp pair tiles])
    qk_proj(0)
    for c in range(KC):
        if c + 1 < KC:
            qk_proj(c + 1)
        drain = pend.pop(0) if len(pend) >= 1 else None
        if drain is not None:
            dov = [ov_psum.tile([DHEAD + 1, 512], F32, name=f"ov{2 * drain[0] + h}",
                                tag="ov", bufs=2) for h in range(2)]
        exps = []
        for k8 in range(8):
            sps = sc_psum.tile([P, 1024], F32, name=f"sps{c}_{k8}", tag="sc", bufs=2)
            for h in range(2):
                nc.tensor.matmul(sps[:, h * 512:(h + 1) * 512],
                                 Kt[c][64 * h:64 * h + 64, k8 * P:(k8 + 1) * P],
                                 Qt[c][64 * h:64 * h + 64, :],
                                 start=True, stop=True, tile_position=(64 * h, 0))
            if k8 % 2 == 0:
                e8 = (epoolA if c % 2 == 0 else epoolB).tile(
                    [P, 2, 1024], F8, name=f"exp{c}_{k8 // 2}", tag="exp")
                exps.append(e8)
            nc.scalar.activation(e8[:, k8 % 2, :], sps[:], AF.Exp, scale=SCALE)
            if drain is not None and k8 % 2 == 1:
                j = k8 // 2
                for h in range(2):
                    nc.tensor.matmul(dov[h][:],
                                     V8[j][:, :, 2 * drain[0] + h, 0:DHEAD + 1],
                                     drain[1][j][:, :, h * 512:(h + 1) * 512],
                                     start=(j == 0), stop=(j == 3), perf_mode=DR)
        if drain is not None:
            finish1(drain[0], dov)
        if c < 3:
            vproj_filler(c)
        pend.append((c, exps))
    while pend:
        pc, exps = pend.pop(0)
        dov = [ov_psum.tile([DHEAD + 1, 512], F32, name=f"ovt{2 * pc + h}",
                            tag="ov", bufs=2) for h in range(2)]
        attnv_dr(pc, exps, dov)
        finish1(pc, dov)

    ln1p.release()
    wqk.release()

    # K2/V2 from context (tiny; drains while the attn tail finishes)
    for mc in range(KC):
        ps = pj_psum.tile([P, 512], F32, name=f"psk2_{mc}", tag="proj", bufs=2)
        for kp in range(KCXP):
            nc.tensor.matmul(ps[:, 0:MCTX], wk2t[mc][:, kp], ctx8[kp][:, :, 0:MCTX],
                             start=(kp == 0), stop=(kp == KCXP - 1), perf_mode=DR)
        nc.vector.tensor_scalar_mul(K2t[mc][:], ps[:, 0:MCTX], WINV)
    nc.gpsimd.memset(V2t[:], 1.0)
    for n0, nsz in ((0, 512), (512, 512), (1024, 256)):
        ps = pj_psum.tile([P, 512], F32, name=f"psv2_{n0}", tag="proj", bufs=2)
        for kp in range(KCXP):
            nc.tensor.matmul(ps[0:MCTX, :nsz], ctx8[kp][:, :, 0:MCTX],
                             wv2s[kp][:, :, n0:n0 + nsz],
                             start=(kp == 0), stop=(kp == KCXP - 1), perf_mode=DR)
        nc.vector.tensor_scalar_mul(
            V2t[:MCTX, n0 // DHEAD:(n0 + nsz) // DHEAD, 0:DHEAD],
            ps[0:MCTX, :nsz].rearrange("p (h e) -> p h e", e=DHEAD), WINV)

    epoolB.release()
    epoolA.release()
    v8p.release()
    qkt.release()
    pj_psum.release()
    ov_psum.release()
    sc_psum.release()

    # ============ phase B: O1 + LN2 + Q2 + attn2 + O2 + LN3 ============

    wffp = tc.alloc_tile_pool(name="wffp", bufs=1, side="right")
    wff1g, wff1a = [], []
    for j in range(JFF):
        wg = wffp.tile([P, KC, P], F16, name=f"wg_{j}", tag="wff1g", bufs=8)
        nc.sync.dma_start(wg[:], d["wff1"][JFF + j])
        wff1g.append(wg)
        wa = wffp.tile([P, KC, P], F16, name=f"wa_{j}", tag="wff1a", bufs=8)
        nc.sync.dma_start(wa[:], d["wff1"][j])
        wff1a.append(wa)

    psB1 = tc.alloc_tile_pool(name="psB1", bufs=1, space="PSUM")

    def proj8_ln(psB, wt_tiles, rhs_pairs, n_kp, consume, ln_idx, x_out, res_tiles,
                 bias0):
        """x_out[mc] = psum/WS (+bias) + res; LN stats inlined; finalize."""
        sums_ps = psB.tile([1, 512], F32, name=f"lns{ln_idx}", tag="lnstat", bufs=2)
        sq_ps = psB.tile([1, 512], F32, name=f"lnq{ln_idx}", tag="lnstat", bufs=2)
        for mc in range(KC):
            ps = psB.tile([P, 512], F32, name=f"ps{ln_idx}_{mc}", tag="proj", bufs=2)
            for kp in range(n_kp):
                nc.tensor.matmul(ps[:], wt_tiles[mc][:, kp], rhs_pairs(kp),
                                 start=(kp == 0), stop=(kp == n_kp - 1), perf_mode=DR)
            if trivial_bias:
                nc.vector.scalar_tensor_tensor(x_out[mc][:], ps[:], WINV,
                                               res_tiles[mc][:], MUL, ADD)
            else:
                xb = tmp.tile([P, T], F32, name=f"xb{ln_idx}_{mc}", tag="xb", bufs=2)
                nc.scalar.activation(xb[:], ps[:], AF.Copy, scale=WINV,
                                     bias=bias_ap(bias0 + mc))
                nc.vector.tensor_add(x_out[mc][:], xb[:], res_tiles[mc][:])
            xsq = tmp.tile([P, T], F16, name=f"xsqB{ln_idx}_{mc}", tag="xsq", bufs=3)
            if mc % 2 == 0:
                nc.vector.tensor_mul(xsq[:], x_out[mc][:], x_out[mc][:])
            else:
                nc.scalar.activation(xsq[:], x_out[mc][:], AF.Square)
            ln_stats_mm(x_out[mc][:], xsq, sums_ps, sq_ps, mc == 0, mc == KC - 1)

        def out(c, mu16, rstd16):
            ln_out(ln_idx, 0, c, x_out[c][:], mu16, rstd16, consume(c))

        ln_finalize(ln_idx, 0, psB, sums_ps, sq_ps, out)

    # x1 is written in place into the residual tiles.
    x1 = resid
    proj8_ln(psB1, wo1t, lambda kp: Ot8[kp][:], KCP,
             lambda c: ln28[c // 2][:, c % 2, :], 1, x1, resid, 0)
    otp.release()
    wpre.release()

    # Q2 projection
    for mc in range(KC):
        ps = psB1.tile([P, 512], F32, name=f"psq2_{mc}", tag="proj", bufs=2)
        for kp in range(KCP):
            nc.tensor.matmul(ps[:], wq2t[mc][:, kp], ln28[kp][:, :, :],
                             start=(kp == 0), stop=(kp == KCP - 1), perf_mode=DR)
        nc.vector.tensor_scalar_mul(Q2t[mc][:], ps[:], WINV)
    ln2p.release()
    wq2p.release()
    psB1.release()

    # attn2: 77 context keys, fp16, single contraction chunk
    psA2 = tc.alloc_tile_pool(name="psA2", bufs=1, space="PSUM")
    e2pool = tc.alloc_tile_pool(name="epool2", bufs=6, side="right")

    def attn2_drain(dc, de):
        dov = [psA2.tile([DHEAD + 1, 512], F32, name=f"ov2_{2 * dc + h}",
                         tag="ov2", bufs=4) for h in range(2)]
        for h in range(2):
            nc.tensor.matmul(dov[h][:], V2t[:MCTX, 2 * dc + h, :],
                             de[:, h * 512:(h + 1) * 512], start=True, stop=True)
        attn_finish(2 * dc, dov[0], O2t8[dc // 2][0:DHEAD, dc % 2, :], psA2, "ov2",
                    ps_bufs=4)
        attn_finish(2 * dc + 1, dov[1], O2t8[dc // 2][DHEAD:2 * DHEAD, dc % 2, :],
                    psA2, "ov2", ps_bufs=4)

    pend2 = []
    for c in range(KC):
        if len(pend2) >= 2:
            attn2_drain(*pend2.pop(0))
        sps = psA2.tile([MCTX, 1024], F32, name=f"sps2_{c}", tag="sc2", bufs=2)
        for h in range(2):
            nc.tensor.matmul(sps[:, h * 512:(h + 1) * 512],
                             K2t[c][64 * h:64 * h + 64, 0:MCTX],
                             Q2t[c][64 * h:64 * h + 64, :],
                             start=True, stop=True, tile_position=(64 * h, 0))
        e = e2pool.tile([MCTX, 1024], F16, name=f"exp2_{c}", tag="exp2")
        nc.scalar.activation(e[:], sps[:], AF.Exp, scale=SCALE)
        pend2.append((c, e))
    while pend2:
        attn2_drain(*pend2.pop(0))
    e2pool.release()
    psA2.release()
    q2p.release()
    k2v2.release()

    # O2 + residual + LN3 (fp16 out feeding the fp16 GEGLU up-proj)
    wo2p = tc.alloc_tile_pool(name="wo2p", bufs=1)       # * (leaked)
    wo2t = []
    for mc in range(KC):
        wt = wo2p.tile([P, KCP, 2, P], F8, name=f"wo2_{mc}", tag="wo2", bufs=KC)
        nc.sync.dma_start(wt[:], d["wo2"][mc])
        wo2t.append(wt)
    x2p = tc.alloc_tile_pool(name="x2p", bufs=1)
    x2 = [x2p.tile([P, T], F16, name=f"x2_{mc}", tag="x2", bufs=KC) for mc in range(KC)]
    ln3p = tc.alloc_tile_pool(name="ln3p", bufs=1)       # * (leaked)
    ln3t = [ln3p.tile([P, T], F16, name=f"ln3_{c}", tag="ln3", bufs=KC)
            for c in range(KC)]
    psB2 = tc.alloc_tile_pool(name="psB2", bufs=1, space="PSUM")
    proj8_ln(psB2, wo2t, lambda kp: O2t8[kp][:], KCP,
             lambda c: ln3t[c][:], 2, x2, x1, 10)
    psB2.release()

    # ============ phase C: GEGLU up-proj (fp16) ============

    hhp = tc.alloc_tile_pool(name="hhp", bufs=1)
    hh8 = [hhp.tile([P, 2, T], F8, name=f"hh8_{i}", tag="hh8", bufs=JFP)
           for i in range(JFP)]

    proj_psum = tc.alloc_tile_pool(name="proj_psum4", bufs=1, space="PSUM")
    for j in range(JFF):
        gps = proj_psum.tile([P, 512], F32, name=f"gps_{j}", tag="proj", bufs=4)
        for kc in range(KC):
            nc.tensor.matmul(gps[:], wff1g[j][:, kc], ln3t[kc][:], start=(kc == 0),
                             stop=(kc == KC - 1))
        gel = tmp.tile([P, T], F16, name=f"gel_{j}", tag="gel", bufs=3)
        if trivial_bias:
            nc.scalar.activation(gel[:], gps[:], AF.Gelu_apprx_tanh)
        else:
            nc.scalar.activation(gel[:], gps[:], AF.Gelu_apprx_tanh,
                                 bias=bias_ap(60 + j))
        aps = proj_psum.tile([P, 512], F32, name=f"aps_{j}", tag="proj", bufs=4)
        for kc in range(KC):
            nc.tensor.matmul(aps[:], wff1a[j][:, kc], ln3t[kc][:], start=(kc == 0),
                             stop=(kc == KC - 1))
        if trivial_bias:
            nc.vector.tensor_mul(hh8[j // 2][:, j % 2, :], aps[:], gel[:])
        else:
            nc.vector.scalar_tensor_tensor(hh8[j // 2][:, j % 2, :], aps[:],
                                           bias_ap(20 + j), gel[:], ADD, MUL)
    wffp.release()

    # ============ phase D: FF down-proj (fp8 DoubleRow) + residual ============

    wf2p = tc.alloc_tile_pool(name="wf2p", bufs=1)
    outp = tc.alloc_tile_pool(name="outp", bufs=4)
    for mc in range(KC):
        wt = wf2p.tile([P, JFP, 2, P], F8, name=f"wff2_{mc}", tag="wff2", bufs=2)
        nc.sync.dma_start(wt[:], d["wff2"][mc])
        ps = proj_psum.tile([P, 512], F32, name=f"psf2_{mc}", tag="proj", bufs=4)
        for kp in range(JFP):
            nc.tensor.matmul(ps[:], wt[:, kp], hh8[kp][:], start=(kp == 0),
                             stop=(kp == JFP - 1), perf_mode=DR)
        ot = outp.tile([P, T], F32, name=f"out_{mc}", tag="out")
        if trivial_bias:
            nc.vector.scalar_tensor_tensor(ot[:], ps[:], WINV, x2[mc][:], MUL, ADD)
        else:
            xb = tmp.tile([P, T], F32, name=f"xbf2_{mc}", tag="xb", bufs=2)
            nc.scalar.activation(xb[:], ps[:], AF.Copy, scale=WINV,
                                 bias=bias_ap(100 + mc))
            nc.vector.tensor_add(ot[:], xb[:], x2[mc][:])
        nc.sync.dma_start(d["out"][mc * P:(mc + 1) * P, :], ot[:])

    outp.release()
    wf2p.release()
    proj_psum.release()
    hhp.release()
    ln3p.release()
    x2p.release()
    wo2p.release()
    o2p.release()
    resp.release()
    ctxp.release()
    tmp.release()
    const.release()


def _q8(w, scale=WS):
    return np.clip(np.asarray(w, np.float32) * scale, -240.0, 240.0).astype(
        ml_dtypes.float8_e4m3)


def _lhst8_layout(w, n_kc, n_mc):
    """[K, M] f32 -> fp8 [n_mc, 128, n_kc/2, 2, 128]: block [mc] is the
    DoubleRow stationary group for output chunk mc (dim -2 pairs two adjacent
    contraction chunks)."""
    a = w.reshape(n_kc // 2, 2, P, n_mc, P).transpose(3, 2, 0, 1, 4)
    return np.ascontiguousarray(_q8(a))


def _rhs8_layout(w, n_kc):
    """[K, M] f32 -> fp8 [n_kc/2, 128, 2, M] moving-operand pair layout."""
    a = w.reshape(n_kc // 2, 2, P, -1).transpose(0, 2, 1, 3)
    return np.ascontiguousarray(_q8(a))


def _lhst_layout(w, n_kc, n_mc):
    """[K, M] f32 -> fp16 [n_mc, 128, n_kc, 128] plain stationary groups."""
    return np.ascontiguousarray(
        w.reshape(n_kc, P, n_mc, P).transpose(2, 1, 0, 3).astype(np.float16))


def _ctx8_layout(ctx):
    """[MCTX, CTX_DIM] f32 -> fp8 [KCXP, 128, 2, MPAD] feature-pair layout."""
    a = np.zeros((KCXP, P, 2, MPAD), np.float32)
    a[:, :, :, :MCTX] = ctx.T.reshape(KCXP, 2, P, MCTX).transpose(0, 2, 1, 3)
    return _q8(a, 1.0)


_BUILT = {}


def _build(trivial_aff, trivial_bias):
    key = (trivial_aff, trivial_bias)
    if key in _BUILT:
        return _BUILT[key]
    nc = bacc.Bacc("TRN2", target_bir_lowering=False, debug=False, num_devices=N_CORES)
    d = {
        "xt": nc.dram_tensor("xt", [DIM, TKV], F16, kind="ExternalInput").ap(),
        "ctx8": nc.dram_tensor("ctx8", [KCXP, P, 2, MPAD], F8, kind="ExternalInput").ap(),
        "wq1": nc.dram_tensor("wq1", [KC, P, KCP, 2, P], F8, kind="ExternalInput").ap(),
        "wk1": nc.dram_tensor("wk1", [KC, P, KCP, 2, P], F8, kind="ExternalInput").ap(),
        "wv1": nc.dram_tensor("wv1", [KCP, P, 2, DIM], F8, kind="ExternalInput").ap(),
        "wo1": nc.dram_tensor("wo1", [KC, P, KCP, 2, P], F8, kind="ExternalInput").ap(),
        "wq2": nc.dram_tensor("wq2", [KC, P, KCP, 2, P], F8, kind="ExternalInput").ap(),
        "wk2": nc.dram_tensor("wk2", [KC, P, KCXP, 2, P], F8, kind="ExternalInput").ap(),
        "wv2": nc.dram_tensor("wv2", [KCXP, P, 2, DIM], F8, kind="ExternalInput").ap(),
        "wo2": nc.dram_tensor("wo2", [KC, P, KCP, 2, P], F8, kind="ExternalInput").ap(),
        "wff1": nc.dram_tensor("wff1", [2 * JFF, P, KC, P], F16, kind="ExternalInput").ap(),
        "wff2": nc.dram_tensor("wff2", [KC, P, JFP, 2, P], F8, kind="ExternalInput").ap(),
        "out": nc.dram_tensor("out", [DIM, T], F32, kind="ExternalOutput").ap(),
    }
    if not trivial_aff:
        d["aff"] = nc.dram_tensor("aff", [P, 60], F32, kind="ExternalInput").ap()
    if not trivial_bias:
        d["biases"] = nc.dram_tensor("biases", [P, 110], F32, kind="ExternalInput").ap()
    with tile.TileContext(nc, pool_alloc_mode="queue") as tc:
        _emit(tc, d, trivial_aff, trivial_bias)
    nc.compile()
    _BUILT[key] = nc
    return nc


def kernel(x, context,
           g1, be1, wq1, wk1, wv1, wo1, bo1,
           g2, be2, wq2, wk2, wv2, wo2, bo2,
           g3, be3, w_ff1, b_ff1, w_ff2, b_ff2,
           _trace=False):
    global last_exec_time_ns
    x = np.asarray(x, np.float32)
    context = np.asarray(context, np.float32)

    affs = [np.asarray(a, np.float32) for a in (g1, be1, g2, be2, g3, be3)]
    biases = [np.asarray(b, np.float32) for b in (bo1, bo2, b_ff1, b_ff2)]
    trivial_aff = all(np.all(a == (1.0 if i % 2 == 0 else 0.0))
                      for i, a in enumerate(affs))
    trivial_bias = all(np.all(b == 0.0) for b in biases)

    nc = _build(trivial_aff, trivial_bias)

    shared = {
        "wq1": _lhst8_layout(np.asarray(wq1, np.float32), KC, KC),
        "wk1": _lhst8_layout(np.asarray(wk1, np.float32), KC, KC),
        "wv1": _rhs8_layout(np.asarray(wv1, np.float32), KC),
        "wo1": _lhst8_layout(np.asarray(wo1, np.float32), KC, KC),
        "wq2": _lhst8_layout(np.asarray(wq2, np.float32), KC, KC),
        "wk2": _lhst8_layout(np.asarray(wk2, np.float32), KCX, KC),
        "wv2": _rhs8_layout(np.asarray(wv2, np.float32), KCX),
        "wo2": _lhst8_layout(np.asarray(wo2, np.float32), KC, KC),
        "wff1": _lhst_layout(np.asarray(w_ff1, np.float32), KC, 2 * JFF),
        "wff2": _lhst8_layout(np.asarray(w_ff2, np.float32), JFF, KC),
    }
    if not trivial_aff:
        aff = np.zeros([P, 60], np.float32)
        for i, a in enumerate(affs):
            # col = ln_idx*20 + (0 for g / 10 for be) + chunk
            ln_idx, j = i // 2, i % 2
            aff[:, ln_idx * 20 + j * 10: ln_idx * 20 + j * 10 + 10] = \
                a.reshape(KC, P).T
        shared["aff"] = aff
    if not trivial_bias:
        bb = np.zeros([P, 110], np.float32)
        bb[:, 0:10] = biases[0].reshape(KC, P).T
        bb[:, 10:20] = biases[1].reshape(KC, P).T
        bb[:, 20:100] = biases[2].reshape(2 * JFF, P).T
        bb[:, 100:110] = biases[3].reshape(KC, P).T
        shared["biases"] = bb

    in_maps = []
    for b in range(BATCH):
        ctx8 = _ctx8_layout(context[b])
        for h in range(2):
            xr = np.roll(x[b], -h * T, axis=0)
            m = dict(shared)
            m["xt"] = np.ascontiguousarray(xr.T.astype(np.float16))
            m["ctx8"] = ctx8
            in_maps.append(m)

    res = bass_utils.run_bass_kernel_spmd(
        nc, in_maps, core_ids=list(range(N_CORES)), trace=_trace)
    last_exec_time_ns = res.exec_time_ns

    out = np.empty((BATCH, NTOK, DIM), np.float32)
    for b in range(BATCH):
        for h in range(2):
            out[b, h * T:(h + 1) * T, :] = res.results[b * 2 + h]["out"].T
    return out


# revision 57
# speedup vs baseline: 1.1543x; 1.0263x over previous
"""BasicTransformerBlock on 8 TRN2 NeuronCores.

Sharding: data-parallel, core = (batch b in 0..3) x (sequence half h in 0..1).
Each core receives its batch element's full sequence rotated so its local 512
rows come first (softmax over keys is permutation invariant), computes K/V of
attn1 for all 1024 tokens (duplicated across the pair, ~10% extra FLOPs, zero
collectives), and everything else for its 512 local tokens only.

On-chip layout: feature-major activations [features on partitions, tokens on
free axis]. The residual stream, LN math and PSUM accumulation stay fp32.
LayerNorm partition reductions and per-token broadcasts use fp16 ones-matmuls.
Attention softmax denominators come free from a ones-column appended to V.

Precision: Q/K/V/O projections (both attns), attn1's attnV and the FF
down-proj run in fp8 e4m3 with DoubleRow matmuls (two 128-row contraction
chunks per pass, 2x PE rate); weights are pre-scaled x32 on the host and the
PSUM rescaled by 1/32 at evacuation. The GEGLU up-proj (the dominant error
path) stays fp16. Attention scores use fp8 operands at the normal rate.

Schedule: Q/K projections run one head-pair ahead of the score matmuls inside
a single fused LN1+proj+attention phase, so the PE streams continuously while
ACT chases with exp. LN2/LN3 statistics are inlined into the O-projection
consume loops. K2/V2 run at the attn1 tail; FF weights stream in early on
wide rolling pools. SBUF pools use the queue (ring) allocator so each pool
releases as soon as its contents die.
"""

import sys
import types

sys.path.insert(0, "/opt/trn_rl_repo")

# concourse fetches the NTFF profile hook from antenv.axon_hooks, which the
# agent image's antenv stub lacks. Register a shim so trace=True works.
if "antenv.axon_hooks" not in sys.modules:
    _hooks = types.ModuleType("antenv.axon_hooks")
    _HOOK = [None]

    def _get_hook():
        if _HOOK[0] is None:
            try:
                from trn_agent_boot.trn_boot import _ntff_profile_via_ctypes

                _HOOK[0] = _ntff_profile_via_ctypes("/opt/axon/libaxon_pjrt.so")
            except Exception:
                _HOOK[0] = None
        return _HOOK[0]

    _hooks.get_axon_ntff_profile_hook = _get_hook
    _hooks.set_axon_ntff_profile_hook = lambda h: _HOOK.__setitem__(0, h)
    sys.modules["antenv.axon_hooks"] = _hooks
    try:
        import antenv

        antenv.axon_hooks = _hooks
    except ImportError:
        pass

import ml_dtypes
import numpy as np

import concourse.bass as bass
import concourse.mybir as mybir
import concourse.tile as tile
from concourse import bacc, bass_utils

dt = mybir.dt
F32, F16, F8 = dt.float32, dt.float16, dt.float8e4
AF = mybir.ActivationFunctionType
DR = mybir.MatmulPerfMode.DoubleRow
MUL, ADD, SUB = mybir.AluOpType.mult, mybir.AluOpType.add, mybir.AluOpType.subtract

DIM, HEADS, DHEAD, CTX_DIM, DFF = 1280, 20, 64, 768, 5120
BATCH, NTOK, MCTX = 4, 1024, 77
EPS = 1e-5
SCALE = DHEAD ** -0.5
N_CORES = 8
T = 512         # local tokens per core
TKV = 1024      # attn1 key/value tokens per core
KC = DIM // 128           # 10
KCP = KC // 2             # 5 contraction pairs
KCX = CTX_DIM // 128      # 6
KCXP = KCX // 2           # 3
JFF = DFF // 128          # 40 (chunks of the gated hidden)
JFP = JFF // 2            # 20 pairs for the down-proj contraction
P = 128
WS = 32.0                 # fp8 weight pre-scale (power of two)
WINV = 1.0 / WS
VPAD = 68                 # DHEAD+1 padded so the V pair stride is 16B aligned
MPAD = 80                 # MCTX padded likewise for the ctx pair stride

last_exec_time_ns = None


def _emit(tc, d, trivial_aff, trivial_bias):
    nc = tc.nc

    const = tc.alloc_tile_pool(name="const", bufs=1)
    ones_col = const.tile([P, 1], F16, name="ones_col")
    nc.vector.memset(ones_col[:], 1.0)
    ones_row = const.tile([1, P], F16, name="ones_row")
    nc.vector.memset(ones_row[:], 1.0)
    if not trivial_aff:
        aff = const.tile([P, 60], F32, name="aff")
        nc.sync.dma_start(aff[:], d["aff"])
    if not trivial_bias:
        biases = const.tile([P, 110], F32, name="biases")
        nc.sync.dma_start(biases[:], d["biases"])

    tmp = tc.alloc_tile_pool(name="tmp", bufs=1)

    # ---------------- helpers ----------------

    def bias_ap(col):
        return biases[:, col:col + 1]

    def ln_stats_mm(xh_ap, xsq, sums_ps, sq_ps, start, stop):
        nc.tensor.matmul(sums_ps[:], ones_col[:], xh_ap, start=start, stop=stop)
        nc.tensor.matmul(sq_ps[:], ones_col[:], xsq[:], start=start, stop=stop)

    def ln_finalize(ln_idx, t, psum_p, sums_ps, sq_ps, out_fn):
        """Turn accumulated sum / sum-sq PSUMs into mu/rstd and emit the
        normalized outputs via out_fn(c, mu16, rstd16)."""
        ssum = tmp.tile([1, 512], F16, name=f"ssum{ln_idx}_{t}", tag="ssum", bufs=2)
        nc.vector.tensor_copy(out=ssum[:], in_=sums_ps[:])
        ssq = tmp.tile([1, 512], F16, name=f"ssq{ln_idx}_{t}", tag="ssq", bufs=2)
        nc.vector.tensor_copy(out=ssq[:], in_=sq_ps[:])
        bs_ps = psum_p.tile([P, 512], F32, name=f"bs{ln_idx}_{t}", tag="proj", bufs=2)
        nc.tensor.matmul(bs_ps[:], ones_row[:], ssum[:], start=True, stop=True)
        bq_ps = psum_p.tile([P, 512], F32, name=f"bq{ln_idx}_{t}", tag="proj", bufs=2)
        nc.tensor.matmul(bq_ps[:], ones_row[:], ssq[:], start=True, stop=True)
        mu16 = tmp.tile([P, 512], F16, name=f"mu16_{ln_idx}_{t}", tag="mu16", bufs=2)
        nc.vector.tensor_scalar_mul(mu16[:], bs_ps[:], 1.0 / DIM)
        musq = tmp.tile([P, 512], F16, name=f"musq{ln_idx}_{t}", tag="musq", bufs=1)
        nc.vector.tensor_mul(musq[:], mu16[:], mu16[:])
        # musq - EPS, so var = ex2 - musq + EPS below
        nc.vector.tensor_scalar_sub(musq[:], musq[:], EPS)
        var = tmp.tile([P, 512], F16, name=f"var{ln_idx}_{t}", tag="var", bufs=1)
        nc.vector.scalar_tensor_tensor(var[:], bq_ps[:], 1.0 / DIM, musq[:], MUL, SUB)
        std = tmp.tile([P, 512], F32, name=f"std{ln_idx}_{t}", tag="std", bufs=1)
        nc.scalar.sqrt(std[:], var[:])
        rstd = tmp.tile([P, 512], F32, name=f"rstd{ln_idx}_{t}", tag="rstd", bufs=1)
        nc.vector.reciprocal_approx_fast(rstd[:], std[:])
        rstd16 = tmp.tile([P, 512], F16, name=f"rstd16_{ln_idx}_{t}", tag="rstd16",
                          bufs=2)
        nc.vector.tensor_copy(out=rstd16[:], in_=rstd[:])
        for c in range(KC):
            out_fn(c, mu16, rstd16)

    def ln_out(ln_idx, t, c, src_ap, mu16, rstd16, out_ap, eng=None):
        """out = (src - mu) * rstd (* g + b)."""
        eng = eng or nc.vector
        xm = tmp.tile([P, 512], F16, name=f"xm{ln_idx}_{t}_{c}", tag="xm", bufs=4)
        eng.tensor_sub(xm[:], src_ap, mu16[:])
        if trivial_aff:
            eng.tensor_mul(out_ap, xm[:], rstd16[:])
        else:
            xn = tmp.tile([P, 512], F16, name=f"xn{ln_idx}_{t}_{c}", tag="xn", bufs=3)
            nc.vector.tensor_mul(xn[:], xm[:], rstd16[:])
            g_ap = aff[:, ln_idx * 20 + c: ln_idx * 20 + c + 1]
            be_ap = aff[:, ln_idx * 20 + 10 + c: ln_idx * 20 + 10 + c + 1]
            xg = tmp.tile([P, 512], F16, name=f"xg{ln_idx}_{t}_{c}", tag="xg", bufs=3)
            nc.vector.tensor_scalar_mul(xg[:], xn[:], g_ap)
            nc.scalar.activation(out_ap, xg[:], AF.Copy, bias=be_ap)

    def attn_finish(head, ops_, out_ap, psum_p, ps_tag, evac_act=False, ps_bufs=2):
        usb = tmp.tile([DHEAD + 1, 512], F16, name=f"usb{head}", tag="usb", bufs=2)
        if evac_act:
            nc.scalar.copy(usb[:], ops_[:])
        else:
            nc.vector.tensor_copy(out=usb[:], in_=ops_[:])
        den = tmp.tile([1, 512], F32, name=f"den{head}", tag="den", bufs=2)
        if evac_act:
            nc.scalar.copy(den[:], ops_[DHEAD:DHEAD + 1, :])
        else:
            nc.vector.tensor_copy(out=den[:], in_=ops_[DHEAD:DHEAD + 1, :])
        rec32 = tmp.tile([1, 512], F32, name=f"rec32_{head}", tag="rec32", bufs=2)
        nc.vector.reciprocal_approx_fast(rec32[:], den[:])
        rec = tmp.tile([1, 512], F16, name=f"rec{head}", tag="rec", bufs=2)
        if evac_act:
            nc.scalar.copy(rec[:], rec32[:])
        else:
            nc.vector.tensor_copy(out=rec[:], in_=rec32[:])
        bps = psum_p.tile([DHEAD, 512], F32, name=f"bps{head}", tag=ps_tag,
                          bufs=ps_bufs)
        nc.tensor.matmul(bps[:], ones_row[:, :DHEAD], rec[:], start=True, stop=True)
        nc.vector.tensor_mul(out_ap, usb[:DHEAD, :], bps[:])

    # ============ phase A: LN1 + QKV projections + attn1, fused ============
    # Left-side pools open in death-descending order (LIFO release discipline;
    # pools marked * stay open to the end: the queue allocator makes the dead
    # space explicit and cheap).

    # x DMA is the critical path to LN1 — allocate xpool first and emit its
    # DMAs before any weight DMA so the DMA queues serve it first.
    xpool = tc.alloc_tile_pool(name="xpool", bufs=1, side="right")
    x_sb = []
    for c in range(KC):
        xc = xpool.tile([P, TKV], F16, name=f"x_{c}", tag="x", bufs=KC)
        nc.sync.dma_start(xc[:], d["xt"][c * P:(c + 1) * P, :])
        x_sb.append(xc)

    ctxp = tc.alloc_tile_pool(name="ctxp", bufs=1)          # * (tiny)
    ctx8 = []
    for kp in range(KCXP):
        c8 = ctxp.tile([P, 2, MPAD], F8, name=f"ctx8_{kp}", tag="ctx8", bufs=KCXP)
        nc.sync.dma_start(c8[:], d["ctx8"][kp])
        ctx8.append(c8)

    # Residual slice of x; O1 writes x1 into these tiles in place.      *
    resp = tc.alloc_tile_pool(name="resp", bufs=1)
    resid = [resp.tile([P, T], F16, name=f"res_{c}", tag="res", bufs=KC)
             for c in range(KC)]

    o2p = tc.alloc_tile_pool(name="o2p", bufs=1)            # * (attn2 out)
    O2t8 = [o2p.tile([P, 2, T], F8, name=f"o2t8_{i}", tag="o2t8", bufs=KCP)
            for i in range(KCP)]

    k2v2 = tc.alloc_tile_pool(name="k2v2", bufs=1)          # dies at attn2
    K2t = [k2v2.tile([P, MCTX], F16, name=f"k2t_{mc}", tag="k2t", bufs=KC)
           for mc in range(KC)]
    V2t = k2v2.tile([P, HEADS, DHEAD + 1], F16, name="v2t", tag="v2t", bufs=1)

    q2p = tc.alloc_tile_pool(name="q2p", bufs=1)            # dies at attn2
    Q2t = [q2p.tile([P, T], F8, name=f"q2t_{mc}", tag="q2t", bufs=KC)
           for mc in range(KC)]

    wq2p = tc.alloc_tile_pool(name="wq2p", bufs=1)          # dies after Q2
    wq2t = [wq2p.tile([P, KCP, 2, P], F8, name=f"wq2_{mc}", tag="wq2", bufs=KC)
            for mc in range(KC)]

    ln2p = tc.alloc_tile_pool(name="ln2p", bufs=1)          # dies after Q2
    ln28 = [ln2p.tile([P, 2, T], F8, name=f"ln28_{i}", tag="ln28", bufs=KCP)
            for i in range(KCP)]

    wpre = tc.alloc_tile_pool(name="wpre", bufs=1)          # dies after O1
    wo1t = [wpre.tile([P, KCP, 2, P], F8, name=f"wo1_{mc}", tag="wo1", bufs=KC)
            for mc in range(KC)]
    wk2t = [wpre.tile([P, KCXP, 2, P], F8, name=f"wk2_{mc}", tag="wk2", bufs=KC)
            for mc in range(KC)]
    wv2s = [wpre.tile([P, 2, DIM], F8, name=f"wv2_{kp}", tag="wv2", bufs=KCXP)
            for kp in range(KCXP)]

    otp = tc.alloc_tile_pool(name="otp", bufs=1)            # dies after O1
    Ot8 = [otp.tile([P, 2, T], F8, name=f"ot8_{i}", tag="ot8", bufs=KCP)
           for i in range(KCP)]

    wqk = tc.alloc_tile_pool(name="wqk", bufs=1)            # dies at pipe end
    wq1t, wk1t, wv1t = [], [], []
    for mc in range(KC):
        wq1t.append(wqk.tile([P, KCP, 2, P], F8, name=f"wq1_{mc}", tag="wq1", bufs=KC))
        wk1t.append(wqk.tile([P, KCP, 2, P], F8, name=f"wk1_{mc}", tag="wk1", bufs=KC))
    wv1t = [wqk.tile([P, 2, DIM], F8, name=f"wv1_{kp}", tag="wv1", bufs=KCP)
            for kp in range(KCP)]

    ln1p = tc.alloc_tile_pool(name="ln1p", bufs=1)          # dies at pipe end
    ln18 = [ln1p.tile([P, 2, TKV], F8, name=f"ln18_{i}", tag="ln18", bufs=KCP)
            for i in range(KCP)]

    # Weight DMAs, in order of first use (after x so they can't delay LN1).
    for mc in range(KC):
        nc.sync.dma_start(wq1t[mc][:], d["wq1"][mc])
        nc.sync.dma_start(wk1t[mc][:], d["wk1"][mc])
    for kp in range(KCP):
        nc.sync.dma_start(wv1t[kp][:], d["wv1"][kp])
    for c in range(KC):
        nc.sync.dma_start(resid[c][:], d["xt"][c * P:(c + 1) * P, 0:T])
    for mc in range(KC):
        nc.sync.dma_start(wk2t[mc][:], d["wk2"][mc])
    for kp in range(KCXP):
        nc.sync.dma_start(wv2s[kp][:], d["wv2"][kp])
    for mc in range(KC):
        nc.sync.dma_start(wo1t[mc][:], d["wo1"][mc])
    for mc in range(KC):
        nc.sync.dma_start(wq2t[mc][:], d["wq2"][mc])

    # LN1 over all 1024 keys. x arrives fp16 and feeds the stats matmuls
    # directly; both 512-token tiles' stats run back-to-back on the PE, with
    # the normalize work split across DVE and GpSimd.
    ln_psum = tc.alloc_tile_pool(name="ln_psum", bufs=1, space="PSUM")
    stats = []
    for t in range(2):
        sl = slice(t * 512, (t + 1) * 512)
        sums_ps = ln_psum.tile([1, 512], F32, name=f"lns0_{t}", tag="lnstat", bufs=4)
        sq_ps = ln_psum.tile([1, 512], F32, name=f"lnq0_{t}", tag="lnstat", bufs=4)
        for c in range(KC):
            xsq = tmp.tile([P, 512], F16, name=f"xsq0_{t}_{c}", tag="xsq", bufs=3)
            if c % 2 == 0:
                nc.vector.tensor_mul(xsq[:], x_sb[c][:, sl], x_sb[c][:, sl])
            else:
                nc.scalar.activation(xsq[:], x_sb[c][:, sl], AF.Square)
            ln_stats_mm(x_sb[c][:, sl], xsq, sums_ps, sq_ps, c == 0, c == KC - 1)
        stats.append((sums_ps, sq_ps))
    for t in range(2):
        sl = slice(t * 512, (t + 1) * 512)

        def ln1_out(c, mu16, rstd16, t=t, sl=sl):
            ln_out(0, t, c, x_sb[c][:, sl], mu16, rstd16,
                   ln18[c // 2][:, c % 2, sl],
                   eng=(nc.gpsimd if c % 3 == 2 else nc.vector))

        ln_finalize(0, t, ln_psum, stats[t][0], stats[t][1], ln1_out)
    xpool.release()
    ln_psum.release()

    sc_psum = tc.alloc_tile_pool(name="sc_psum", bufs=1, space="PSUM")
    ov_psum = tc.alloc_tile_pool(name="ov_psum", bufs=1, space="PSUM")
    pj_psum = tc.alloc_tile_pool(name="pj_psum", bufs=1, space="PSUM")
    qkt = tc.alloc_tile_pool(name="qkt", bufs=1, side="right")
    v8p = tc.alloc_tile_pool(name="v8p", bufs=1, side="right")
    epoolA = tc.alloc_tile_pool(name="epoolA", bufs=4, side="right")
    epoolB = tc.alloc_tile_pool(name="epoolB", bufs=4, side="right")

    Qt = [qkt.tile([P, T], F8, name=f"qt_{mc}", tag="qt", bufs=KC) for mc in range(KC)]
    Kt = [qkt.tile([P, TKV], F8, name=f"kt_{mc}", tag="kt", bufs=KC) for mc in range(KC)]
    V8 = [v8p.tile([P, 2, HEADS, VPAD], F8, name=f"v8_{j}", tag="v8", bufs=4)
          for j in range(4)]

    def qk_proj(c):
        """Project Q chunk c (local tokens) and K chunk c (all 1024 keys)."""
        ps = pj_psum.tile([P, 512], F32, name=f"psq_{c}", tag="proj", bufs=2)
        for kp in range(KCP):
            nc.tensor.matmul(ps[:], wq1t[c][:, kp], ln18[kp][:, :, 0:T],
                             start=(kp == 0), stop=(kp == KCP - 1), perf_mode=DR)
        nc.vector.tensor_scalar_mul(Qt[c][:], ps[:], WINV)
        for th in range(2):
            sl = slice(th * 512, (th + 1) * 512)
            ps = pj_psum.tile([P, 512], F32, name=f"psk_{c}_{th}", tag="proj", bufs=2)
            for kp in range(KCP):
                nc.tensor.matmul(ps[:], wk1t[c][:, kp], ln18[kp][:, :, sl],
                                 start=(kp == 0), stop=(kp == KCP - 1), perf_mode=DR)
            nc.vector.tensor_scalar_mul(Kt[c][:, sl], ps[:], WINV)

    def vproj_filler(nt):
        n0, nsz = ((0, 512), (512, 512), (1024, 256))[nt]
        if nt == 0:
            for j in range(4):
                nc.gpsimd.memset(V8[j][:], 1.0)
        for t8 in range(8):
            ps = pj_psum.tile([P, 512], F32, name=f"psv_{t8}_{n0}", tag="proj", bufs=2)
            for kp in range(KCP):
                nc.tensor.matmul(ps[:, :nsz], ln18[kp][:, :, t8 * P:(t8 + 1) * P],
                                 wv1t[kp][:, :, n0:n0 + nsz],
                                 start=(kp == 0), stop=(kp == KCP - 1), perf_mode=DR)
            nc.vector.tensor_scalar_mul(
                V8[t8 // 2][:, t8 % 2, n0 // DHEAD:(n0 + nsz) // DHEAD, 0:DHEAD],
                ps[:, :nsz].rearrange("p (h e) -> p h e", e=DHEAD), WINV)

    def attnv_dr(pc, exps, dov):
        for j in range(4):
            for h in range(2):
                nc.tensor.matmul(dov[h][:], V8[j][:, :, 2 * pc + h, 0:DHEAD + 1],
                                 exps[j][:, :, h * 512:(h + 1) * 512],
                                 start=(j == 0), stop=(j == 3), perf_mode=DR)

    def finish1(pc, dov):
        attn_finish(2 * pc, dov[0], Ot8[pc // 2][0:DHEAD, pc % 2, :], ov_psum, "ov")
        attn_finish(2 * pc + 1, dov[1], Ot8[pc // 2][DHEAD:2 * DHEAD, pc % 2, :],
                    ov_psum, "ov")

    pend = []  # (pair_idx, [4 exp pair tiles])
    qk_proj(0)
    for c in range(KC):
        if c + 1 < KC:
            qk_proj(c + 1)
        drain = pend.pop(0) if len(pend) >= 1 else None
        if drain is not None:
            dov = [ov_psum.tile([DHEAD + 1, 512], F32, name=f"ov{2 * drain[0] + h}",
                                tag="ov", bufs=2) for h in range(2)]
        exps = []
        for k8 in range(8):
            sps = sc_psum.tile([P, 1024], F32, name=f"sps{c}_{k8}", tag="sc", bufs=2)
            for h in range(2):
                nc.tensor.matmul(sps[:, h * 512:(h + 1) * 512],
                                 Kt[c][64 * h:64 * h + 64, k8 * P:(k8 + 1) * P],
                                 Qt[c][64 * h:64 * h + 64, :],
                                 start=True, stop=True, tile_position=(64 * h, 0))
            if k8 % 2 == 0:
                e8 = (epoolA if c % 2 == 0 else epoolB).tile(
                    [P, 2, 1024], F8, name=f"exp{c}_{k8 // 2}", tag="exp")
                exps.append(e8)
            nc.scalar.activation(e8[:, k8 % 2, :], sps[:], AF.Exp, scale=SCALE)
            if drain is not None and k8 % 2 == 1:
                j = k8 // 2
                for h in range(2):
                    nc.tensor.matmul(dov[h][:],
                                     V8[j][:, :, 2 * drain[0] + h, 0:DHEAD + 1],
                                     drain[1][j][:, :, h * 512:(h + 1) * 512],
                                     start=(j == 0), stop=(j == 3), perf_mode=DR)
        if drain is not None:
            finish1(drain[0], dov)
        if c < 3:
            vproj_filler(c)
        pend.append((c, exps))
    while pend:
        pc, exps = pend.pop(0)
        dov = [ov_psum.tile([DHEAD + 1, 512], F32, name=f"ovt{2 * pc + h}",
                            tag="ov", bufs=2) for h in range(2)]
        attnv_dr(pc, exps, dov)
        finish1(pc, dov)

    ln1p.release()
    wqk.release()

    # K2/V2 from context (tiny; drains while the attn tail finishes)
    for mc in range(KC):
        ps = pj_psum.tile([P, 512], F32, name=f"psk2_{mc}", tag="proj", bufs=2)
        for kp in range(KCXP):
            nc.tensor.matmul(ps[:, 0:MCTX], wk2t[mc][:, kp], ctx8[kp][:, :, 0:MCTX],
                             start=(kp == 0), stop=(kp == KCXP - 1), perf_mode=DR)
        nc.vector.tensor_scalar_mul(K2t[mc][:], ps[:, 0:MCTX], WINV)
    nc.gpsimd.memset(V2t[:], 1.0)
    for n0, nsz in ((0, 512), (512, 512), (1024, 256)):
        ps = pj_psum.tile([P, 512], F32, name=f"psv2_{n0}", tag="proj", bufs=2)
        for kp in range(KCXP):
            nc.tensor.matmul(ps[0:MCTX, :nsz], ctx8[kp][:, :, 0:MCTX],
                             wv2s[kp][:, :, n0:n0 + nsz],
                             start=(kp == 0), stop=(kp == KCXP - 1), perf_mode=DR)
        nc.vector.tensor_scalar_mul(
            V2t[:MCTX, n0 // DHEAD:(n0 + nsz) // DHEAD, 0:DHEAD],
            ps[0:MCTX, :nsz].rearrange("p (h e) -> p h e", e=DHEAD), WINV)

    epoolB.release()
    epoolA.release()
    v8p.release()
    qkt.release()
    pj_psum.release()
    ov_psum.release()
    sc_psum.release()

    # ============ phase B: O1 + LN2 + Q2 + attn2 + O2 + LN3 ============

    # O2 weights ahead of the FF1 stream in the DMA queues (O2 runs first).
    wo2p = tc.alloc_tile_pool(name="wo2p", bufs=1, side="right")
    wo2t = []
    for mc in range(KC):
        wt = wo2p.tile([P, KCP, 2, P], F8, name=f"wo2_{mc}", tag="wo2", bufs=KC)
        nc.sync.dma_start(wt[:], d["wo2"][mc])
        wo2t.append(wt)
    wffp = tc.alloc_tile_pool(name="wffp", bufs=1, side="right")
    wff1g, wff1a = [], []
    for j in range(JFF):
        wg = wffp.tile([P, KC, P], F16, name=f"wg_{j}", tag="wff1g", bufs=8)
        nc.sync.dma_start(wg[:], d["wff1"][JFF + j])
        wff1g.append(wg)
        wa = wffp.tile([P, KC, P], F16, name=f"wa_{j}", tag="wff1a", bufs=8)
        nc.sync.dma_start(wa[:], d["wff1"][j])
        wff1a.append(wa)

    psB1 = tc.alloc_tile_pool(name="psB1", bufs=1, space="PSUM")

    def proj8_ln(psB, wt_tiles, rhs_pairs, n_kp, consume, ln_idx, x_out, res_tiles,
                 bias0):
        """x_out[mc] = psum/WS (+bias) + res; LN stats inlined; finalize."""
        sums_ps = psB.tile([1, 512], F32, name=f"lns{ln_idx}", tag="lnstat", bufs=2)
        sq_ps = psB.tile([1, 512], F32, name=f"lnq{ln_idx}", tag="lnstat", bufs=2)
        for mc in range(KC):
            ps = psB.tile([P, 512], F32, name=f"ps{ln_idx}_{mc}", tag="proj", bufs=2)
            for kp in range(n_kp):
                nc.tensor.matmul(ps[:], wt_tiles[mc][:, kp], rhs_pairs(kp),
                                 start=(kp == 0), stop=(kp == n_kp - 1), perf_mode=DR)
            if trivial_bias:
                nc.vector.scalar_tensor_tensor(x_out[mc][:], ps[:], WINV,
                                               res_tiles[mc][:], MUL, ADD)
            else:
                xb = tmp.tile([P, T], F32, name=f"xb{ln_idx}_{mc}", tag="xb", bufs=2)
                nc.scalar.activation(xb[:], ps[:], AF.Copy, scale=WINV,
                                     bias=bias_ap(bias0 + mc))
                nc.vector.tensor_add(x_out[mc][:], xb[:], res_tiles[mc][:])
            xsq = tmp.tile([P, T], F16, name=f"xsqB{ln_idx}_{mc}", tag="xsq", bufs=3)
            if mc % 2 == 0:
                nc.vector.tensor_mul(xsq[:], x_out[mc][:], x_out[mc][:])
            else:
                nc.scalar.activation(xsq[:], x_out[mc][:], AF.Square)
            ln_stats_mm(x_out[mc][:], xsq, sums_ps, sq_ps, mc == 0, mc == KC - 1)

        def out(c, mu16, rstd16):
            ln_out(ln_idx, 0, c, x_out[c][:], mu16, rstd16, consume(c))

        ln_finalize(ln_idx, 0, psB, sums_ps, sq_ps, out)

    # x1 is written in place into the residual tiles.
    x1 = resid
    proj8_ln(psB1, wo1t, lambda kp: Ot8[kp][:], KCP,
             lambda c: ln28[c // 2][:, c % 2, :], 1, x1, resid, 0)
    otp.release()
    wpre.release()

    # Q2 projection
    for mc in range(KC):
        ps = psB1.tile([P, 512], F32, name=f"psq2_{mc}", tag="proj", bufs=2)
        for kp in range(KCP):
            nc.tensor.matmul(ps[:], wq2t[mc][:, kp], ln28[kp][:, :, :],
                             start=(kp == 0), stop=(kp == KCP - 1), perf_mode=DR)
        nc.vector.tensor_scalar_mul(Q2t[mc][:], ps[:], WINV)
    ln2p.release()
    wq2p.release()
    psB1.release()

    # attn2: 77 context keys, fp16, single contraction chunk
    psA2 = tc.alloc_tile_pool(name="psA2", bufs=1, space="PSUM")
    e2pool = tc.alloc_tile_pool(name="epool2", bufs=6, side="right")

    def attn2_drain(dc, de, recs):
        """attnV + finish for pair dc; the reciprocal denominators were
        computed right after the exps, so nothing here waits on DVE."""
        dov = [psA2.tile([DHEAD, 512], F32, name=f"ov2_{2 * dc + h}",
                         tag="ov2", bufs=4) for h in range(2)]
        for h in range(2):
            nc.tensor.matmul(dov[h][:], V2t[:MCTX, 2 * dc + h, 0:DHEAD],
                             de[:, h * 512:(h + 1) * 512], start=True, stop=True)
        usbs = []
        for h in range(2):
            usb = tmp.tile([DHEAD, 512], F16, name=f"usb2_{2 * dc + h}",
                           tag="usb", bufs=2)
            nc.vector.tensor_copy(out=usb[:], in_=dov[h][:])
            usbs.append(usb)
        for h in range(2):
            bps = psA2.tile([DHEAD, 512], F32, name=f"bps2_{2 * dc + h}",
                            tag="ov2", bufs=4)
            nc.tensor.matmul(bps[:], ones_row[:, :DHEAD], recs[h][:],
                             start=True, stop=True)
            out_ap = O2t8[dc // 2][h * DHEAD:(h + 1) * DHEAD, dc % 2, :]
            nc.vector.tensor_mul(out_ap, usbs[h][:], bps[:])

    pend2 = []
    for c in range(KC):
        if len(pend2) >= 2:
            attn2_drain(*pend2.pop(0))
        sps = psA2.tile([MCTX, 1024], F32, name=f"sps2_{c}", tag="sc2", bufs=1)
        for h in range(2):
            nc.tensor.matmul(sps[:, h * 512:(h + 1) * 512],
                             K2t[c][64 * h:64 * h + 64, 0:MCTX],
                             Q2t[c][64 * h:64 * h + 64, :],
                             start=True, stop=True, tile_position=(64 * h, 0))
        e = e2pool.tile([MCTX, 1024], F16, name=f"exp2_{c}", tag="exp2")
        nc.scalar.activation(e[:], sps[:], AF.Exp, scale=SCALE)
        recs = []
        for h in range(2):
            dps = psA2.tile([1, 512], F32, name=f"den2_{c}_{h}", tag="den2", bufs=2)
            nc.tensor.matmul(dps[:], ones_col[0:MCTX, :],
                             e[:, h * 512:(h + 1) * 512], start=True, stop=True)
            rec32 = tmp.tile([1, 512], F32, name=f"r32_{c}_{h}", tag="rec32", bufs=2)
            nc.vector.reciprocal_approx_fast(rec32[:], dps[:])
            rec = tmp.tile([1, 512], F16, name=f"r16_{c}_{h}", tag="rec2", bufs=4)
            nc.vector.tensor_copy(out=rec[:], in_=rec32[:])
            recs.append(rec)
        pend2.append((c, e, recs))
    while pend2:
        attn2_drain(*pend2.pop(0))
    e2pool.release()
    psA2.release()
    q2p.release()
    k2v2.release()

    # O2 + residual + LN3 (fp16 out feeding the fp16 GEGLU up-proj)
    x2p = tc.alloc_tile_pool(name="x2p", bufs=1)
    x2 = [x2p.tile([P, T], F16, name=f"x2_{mc}", tag="x2", bufs=KC) for mc in range(KC)]
    ln3p = tc.alloc_tile_pool(name="ln3p", bufs=1)       # * (leaked)
    ln3t = [ln3p.tile([P, T], F16, name=f"ln3_{c}", tag="ln3", bufs=KC)
            for c in range(KC)]
    psB2 = tc.alloc_tile_pool(name="psB2", bufs=1, space="PSUM")
    proj8_ln(psB2, wo2t, lambda kp: O2t8[kp][:], KCP,
             lambda c: ln3t[c][:], 2, x2, x1, 10)
    psB2.release()

    # ============ phase C: GEGLU up-proj (fp16) ============

    hhp = tc.alloc_tile_pool(name="hhp", bufs=1)
    hh8 = [hhp.tile([P, 2, T], F8, name=f"hh8_{i}", tag="hh8", bufs=JFP)
           for i in range(JFP)]

    proj_psum = tc.alloc_tile_pool(name="proj_psum4", bufs=1, space="PSUM")
    for j in range(JFF):
        gps = proj_psum.tile([P, 512], F32, name=f"gps_{j}", tag="proj", bufs=4)
        for kc in range(KC):
            nc.tensor.matmul(gps[:], wff1g[j][:, kc], ln3t[kc][:], start=(kc == 0),
                             stop=(kc == KC - 1))
        gel = tmp.tile([P, T], F16, name=f"gel_{j}", tag="gel", bufs=3)
        if trivial_bias:
            nc.scalar.activation(gel[:], gps[:], AF.Gelu_apprx_tanh)
        else:
            nc.scalar.activation(gel[:], gps[:], AF.Gelu_apprx_tanh,
                                 bias=bias_ap(60 + j))
        aps = proj_psum.tile([P, 512], F32, name=f"aps_{j}", tag="proj", bufs=4)
        for kc in range(KC):
            nc.tensor.matmul(aps[:], wff1a[j][:, kc], ln3t[kc][:], start=(kc == 0),
                             stop=(kc == KC - 1))
        if trivial_bias:
            nc.vector.tensor_mul(hh8[j // 2][:, j % 2, :], aps[:], gel[:])
        else:
            nc.vector.scalar_tensor_tensor(hh8[j // 2][:, j % 2, :], aps[:],
                                           bias_ap(20 + j), gel[:], ADD, MUL)
    wffp.release()

    # ============ phase D: FF down-proj (fp8 DoubleRow) + residual ============

    wf2p = tc.alloc_tile_pool(name="wf2p", bufs=1)
    outp = tc.alloc_tile_pool(name="outp", bufs=4)
    for mc in range(KC):
        wt = wf2p.tile([P, JFP, 2, P], F8, name=f"wff2_{mc}", tag="wff2", bufs=2)
        nc.sync.dma_start(wt[:], d["wff2"][mc])
        ps = proj_psum.tile([P, 512], F32, name=f"psf2_{mc}", tag="proj", bufs=4)
        for kp in range(JFP):
            nc.tensor.matmul(ps[:], wt[:, kp], hh8[kp][:], start=(kp == 0),
                             stop=(kp == JFP - 1), perf_mode=DR)
        ot = outp.tile([P, T], F32, name=f"out_{mc}", tag="out")
        if trivial_bias:
            nc.vector.scalar_tensor_tensor(ot[:], ps[:], WINV, x2[mc][:], MUL, ADD)
        else:
            xb = tmp.tile([P, T], F32, name=f"xbf2_{mc}", tag="xb", bufs=2)
            nc.scalar.activation(xb[:], ps[:], AF.Copy, scale=WINV,
                                 bias=bias_ap(100 + mc))
            nc.vector.tensor_add(ot[:], xb[:], x2[mc][:])
        nc.sync.dma_start(d["out"][mc * P:(mc + 1) * P, :], ot[:])

    outp.release()
    wf2p.release()
    proj_psum.release()
    hhp.release()
    ln3p.release()
    x2p.release()
    wo2p.release()
    o2p.release()
    resp.release()
    ctxp.release()
    tmp.release()
    const.release()


def _q8(w, scale=WS):
    return np.clip(np.asarray(w, np.float32) * scale, -240.0, 240.0).astype(
        ml_dtypes.float8_e4m3)


def _lhst8_layout(w, n_kc, n_mc):
    """[K, M] f32 -> fp8 [n_mc, 128, n_kc/2, 2, 128]: block [mc] is the
    DoubleRow stationary group for output chunk mc (dim -2 pairs two adjacent
    contraction chunks)."""
    a = w.reshape(n_kc // 2, 2, P, n_mc, P).transpose(3, 2, 0, 1, 4)
    return np.ascontiguousarray(_q8(a))


def _rhs8_layout(w, n_kc):
    """[K, M] f32 -> fp8 [n_kc/2, 128, 2, M] moving-operand pair layout."""
    a = w.reshape(n_kc // 2, 2, P, -1).transpose(0, 2, 1, 3)
    return np.ascontiguousarray(_q8(a))


def _lhst_layout(w, n_kc, n_mc):
    """[K, M] f32 -> fp16 [n_mc, 128, n_kc, 128] plain stationary groups."""
    return np.ascontiguousarray(
        w.reshape(n_kc, P, n_mc, P).transpose(2, 1, 0, 3).astype(np.float16))


def _ctx8_layout(ctx):
    """[MCTX, CTX_DIM] f32 -> fp8 [KCXP, 128, 2, MPAD] feature-pair layout."""
    a = np.zeros((KCXP, P, 2, MPAD), np.float32)
    a[:, :, :, :MCTX] = ctx.T.reshape(KCXP, 2, P, MCTX).transpose(0, 2, 1, 3)
    return _q8(a, 1.0)


_BUILT = {}


def _build(trivial_aff, trivial_bias):
    key = (trivial_aff, trivial_bias)
    if key in _BUILT:
        return _BUILT[key]
    nc = bacc.Bacc("TRN2", target_bir_lowering=False, debug=False, num_devices=N_CORES)
    d = {
        "xt": nc.dram_tensor("xt", [DIM, TKV], F16, kind="ExternalInput").ap(),
        "ctx8": nc.dram_tensor("ctx8", [KCXP, P, 2, MPAD], F8, kind="ExternalInput").ap(),
        "wq1": nc.dram_tensor("wq1", [KC, P, KCP, 2, P], F8, kind="ExternalInput").ap(),
        "wk1": nc.dram_tensor("wk1", [KC, P, KCP, 2, P], F8, kind="ExternalInput").ap(),
        "wv1": nc.dram_tensor("wv1", [KCP, P, 2, DIM], F8, kind="ExternalInput").ap(),
        "wo1": nc.dram_tensor("wo1", [KC, P, KCP, 2, P], F8, kind="ExternalInput").ap(),
        "wq2": nc.dram_tensor("wq2", [KC, P, KCP, 2, P], F8, kind="ExternalInput").ap(),
        "wk2": nc.dram_tensor("wk2", [KC, P, KCXP, 2, P], F8, kind="ExternalInput").ap(),
        "wv2": nc.dram_tensor("wv2", [KCXP, P, 2, DIM], F8, kind="ExternalInput").ap(),
        "wo2": nc.dram_tensor("wo2", [KC, P, KCP, 2, P], F8, kind="ExternalInput").ap(),
        "wff1": nc.dram_tensor("wff1", [2 * JFF, P, KC, P], F16, kind="ExternalInput").ap(),
        "wff2": nc.dram_tensor("wff2", [KC, P, JFP, 2, P], F8, kind="ExternalInput").ap(),
        "out": nc.dram_tensor("out", [DIM, T], F32, kind="ExternalOutput").ap(),
    }
    if not trivial_aff:
        d["aff"] = nc.dram_tensor("aff", [P, 60], F32, kind="ExternalInput").ap()
    if not trivial_bias:
        d["biases"] = nc.dram_tensor("biases", [P, 110], F32, kind="ExternalInput").ap()
    with tile.TileContext(nc, pool_alloc_mode="queue") as tc:
        _emit(tc, d, trivial_aff, trivial_bias)
    nc.compile()
    _BUILT[key] = nc
    return nc


def kernel(x, context,
           g1, be1, wq1, wk1, wv1, wo1, bo1,
           g2, be2, wq2, wk2, wv2, wo2, bo2,
           g3, be3, w_ff1, b_ff1, w_ff2, b_ff2,
           _trace=False):
    global last_exec_time_ns
    x = np.asarray(x, np.float32)
    context = np.asarray(context, np.float32)

    affs = [np.asarray(a, np.float32) for a in (g1, be1, g2, be2, g3, be3)]
    biases = [np.asarray(b, np.float32) for b in (bo1, bo2, b_ff1, b_ff2)]
    trivial_aff = all(np.all(a == (1.0 if i % 2 == 0 else 0.0))
                      for i, a in enumerate(affs))
    trivial_bias = all(np.all(b == 0.0) for b in biases)

    nc = _build(trivial_aff, trivial_bias)

    shared = {
        "wq1": _lhst8_layout(np.asarray(wq1, np.float32), KC, KC),
        "wk1": _lhst8_layout(np.asarray(wk1, np.float32), KC, KC),
        "wv1": _rhs8_layout(np.asarray(wv1, np.float32), KC),
        "wo1": _lhst8_layout(np.asarray(wo1, np.float32), KC, KC),
        "wq2": _lhst8_layout(np.asarray(wq2, np.float32), KC, KC),
        "wk2": _lhst8_layout(np.asarray(wk2, np.float32), KCX, KC),
        "wv2": _rhs8_layout(np.asarray(wv2, np.float32), KCX),
        "wo2": _lhst8_layout(np.asarray(wo2, np.float32), KC, KC),
        "wff1": _lhst_layout(np.asarray(w_ff1, np.float32), KC, 2 * JFF),
        "wff2": _lhst8_layout(np.asarray(w_ff2, np.float32), JFF, KC),
    }
    if not trivial_aff:
        aff = np.zeros([P, 60], np.float32)
        for i, a in enumerate(affs):
            # col = ln_idx*20 + (0 for g / 10 for be) + chunk
            ln_idx, j = i // 2, i % 2
            aff[:, ln_idx * 20 + j * 10: ln_idx * 20 + j * 10 + 10] = \
                a.reshape(KC, P).T
        shared["aff"] = aff
    if not trivial_bias:
        bb = np.zeros([P, 110], np.float32)
        bb[:, 0:10] = biases[0].reshape(KC, P).T
        bb[:, 10:20] = biases[1].reshape(KC, P).T
        bb[:, 20:100] = biases[2].reshape(2 * JFF, P).T
        bb[:, 100:110] = biases[3].reshape(KC, P).T
        shared["biases"] = bb

    in_maps = []
    for b in range(BATCH):
        ctx8 = _ctx8_layout(context[b])
        for h in range(2):
            xr = np.roll(x[b], -h * T, axis=0)
            m = dict(shared)
            m["xt"] = np.ascontiguousarray(xr.T.astype(np.float16))
            m["ctx8"] = ctx8
            in_maps.append(m)

    res = bass_utils.run_bass_kernel_spmd(
        nc, in_maps, core_ids=list(range(N_CORES)), trace=_trace)
    last_exec_time_ns = res.exec_time_ns

    out = np.empty((BATCH, NTOK, DIM), np.float32)
    for b in range(BATCH):
        for h in range(2):
            out[b, h * T:(h + 1) * T, :] = res.results[b * 2 + h]["out"].T
    return out
